# revision 19
# baseline (speedup 1.0000x reference)
"""Multi-head-free attention (softmax over the QUERY axis) on 8 trn2 NeuronCores.

Problem: x:[4,2048,1024], Wq/Wk/Wv:[1024,1024], bq/bk/bv:[1024]
    q = x@Wq+bq ; k = x@Wk+bk ; v = x@Wv+bv
    scores = einsum('bqd,bkd->bqk', q, k) / 32
    attn   = softmax(scores, axis=1)          # over q (dim 1)!
    out    = einsum('bqk,bkv->bqv', attn, v)

Sharding: 4 batches x 2-way split of the KEY axis across 8 cores
(core c -> batch c//2, key-half c%2).  Because softmax normalizes over
q for each fixed k, a k-split keeps the softmax fully local per core.
Each core computes a partial out[q, dv] summed over its k-half; a
2-core ReduceScatter (over the q axis) completes the sum, and rank r of
each pair returns q-rows [r*1024, (r+1)*1024) of its batch.

All matmuls run as float32r (full PE rate at N=512 moving dim, fp32
storage).  The attn*V contraction runs in bf16 (attn weights + V), with
fp32 PSUM accumulation.
"""

import sys

if "/opt/trn_rl_repo" not in sys.path:
    sys.path.insert(0, "/opt/trn_rl_repo")

import numpy as np

P = 128  # SBUF partitions


class Cfg:
    def __init__(self, B=4, S=2048, E=1024, D=1024, NB=512, n_cores=8, mm="f32r"):
        self.B, self.S, self.E, self.D, self.NB = B, S, E, D, NB
        self.mm = mm
        self.SH = S // 2          # per-core key-half length
        self.NE = E // P          # e (contraction) tiles
        self.ND = D // P          # d tiles
        self.NQB = S // NB        # q 512-blocks (full)
        self.NKB = self.SH // NB  # k 512-blocks (half)
        self.NKT = self.SH // P   # k 128-tiles (half)
        self.NQT = S // P         # q 128-tiles (full)
        self.NDVB = D // NB       # dv 512-blocks
        self.n_cores = n_cores
        self.groups = [[2 * i, 2 * i + 1] for i in range(n_cores // 2)]
        self.NCH = 4 if self.NQT % 4 == 0 else 1  # RS chunks (must divide NQT)


PROD = Cfg()


def build_nc(cfg: Cfg):
    from concourse import bacc, bass, mybir, tile

    f32 = mybir.dt.float32
    f32r = mybir.dt.float32r
    bf16 = mybir.dt.bfloat16
    AF = mybir.ActivationFunctionType
    X = mybir.AxisListType.X
    ts = bass.ts

    B, S, E, D, NB = cfg.B, cfg.S, cfg.E, cfg.D, cfg.NB
    SH, NE, ND = cfg.SH, cfg.NE, cfg.ND
    NQB, NKB, NKT, NQT, NDVB = cfg.NQB, cfg.NKB, cfg.NKT, cfg.NQT, cfg.NDVB
    inv_sqrt_d = 1.0 / float(np.sqrt(np.float32(D)))

    nc = bacc.Bacc(None, num_devices=cfg.n_cores)
    dt_in = bf16 if cfg.mm == "bf16" else f32r

    # Per-core inputs (host pre-shards / pre-transposes).
    xt_d = nc.declare_dram_parameter("xt", [E, S], dt_in, isOutput=False)
    xth_d = nc.declare_dram_parameter("xth", [E, SH], dt_in, isOutput=False)
    wq_d = nc.declare_dram_parameter("wq", [E, D], dt_in, isOutput=False)
    wk_d = nc.declare_dram_parameter("wk", [E, D], dt_in, isOutput=False)
    wv_d = nc.declare_dram_parameter("wv", [E, D], dt_in, isOutput=False)
    bq_d = nc.declare_dram_parameter("bq", [D, 1], f32, isOutput=False)
    bk_d = nc.declare_dram_parameter("bk", [D, 1], f32, isOutput=False)
    bv_d = nc.declare_dram_parameter("bv", [1, D], dt_in, isOutput=False)
    ones_d = nc.declare_dram_parameter("onesv", [1, P], dt_in, isOutput=False)
    y_d = nc.declare_dram_parameter("y", [SH, D], f32, isOutput=True)

    def r(ap):  # tensors feeding the PE are already dt_in (f32r or bf16)
        return ap

    with tile.TileContext(nc) as tc:
        with (
            tc.tile_pool(name="w", bufs=8 * ND + 6) as w_pool,
            tc.tile_pool(name="x", bufs=min(10, 2 * NE + 2)) as x_pool,
            tc.tile_pool(name="kt", bufs=1) as kt_pool,
            tc.tile_pool(name="qt", bufs=2 * ND - 2) as qt_pool,
            tc.tile_pool(name="pt", bufs=1) as pt_pool,
            tc.tile_pool(name="v", bufs=1) as v_pool,
            tc.tile_pool(name="ob", bufs=3) as out_pool,
            tc.tile_pool(name="small", bufs=1) as small_pool,
            tc.tile_pool(name="ps", bufs=8, space="PSUM") as ps_pool,
            tc.tile_pool(name="dram", bufs=1, space="DRAM") as dram_pool,
        ):
            part_d = dram_pool.tile([S, D], f32)
            rsout_d = dram_pool.tile([SH, D], f32)

            # ---- constants ----
            ones_t = small_pool.tile([1, P], dt_in, tag="ones")
            nc.sync.dma_start(ones_t[:], ones_d[:])
            bv_t = small_pool.tile([1, D], dt_in, tag="bvrow")
            nc.sync.dma_start(bv_t[:], bv_d[:])
            bq_t, bk_t = [], []
            for dt in range(ND):
                bqt = small_pool.tile([P, 1], f32, tag=f"bq{dt}")
                nc.sync.dma_start(bqt[:], bq_d[ts(dt, P), :])
                bq_t.append(bqt)
                bkt = small_pool.tile([P, 1], f32, tag=f"bk{dt}")
                nc.sync.dma_start(bkt[:], bk_d[ts(dt, P), :])
                bk_t.append(bkt)

            # ---- phase K: KT[dt][d_in_tile, k] = K[k, d]  (k = my half) ----
            kt_tiles = []
            for dt in range(ND):
                ktt = kt_pool.tile([P, SH], dt_in, tag=f"kt{dt}", name=f"ktt{dt}")
                kt_tiles.append(ktt)
            wk_t = {}
            for kb in range(NKB):
                xk = []
                for et in range(NE):
                    t = x_pool.tile([P, NB], dt_in, tag="x", name=f"xk{kb}_{et}")
                    nc.sync.dma_start(t[:], xth_d[ts(et, P), ts(kb, NB)])
                    xk.append(t)
                for dt in range(ND):
                    if kb == 0:
                        for et in range(NE):
                            w = w_pool.tile([P, P], dt_in, tag="w", name=f"wk{et}_{dt}")
                            nc.sync.dma_start(w[:], wk_d[ts(et, P), ts(dt, P)])
                            wk_t[(et, dt)] = w
                    ps = ps_pool.tile([P, NB], f32, tag="ps", name="psk")
                    for et in range(NE):
                        nc.tensor.matmul(
                            ps[:], r(wk_t[(et, dt)][:]), r(xk[et][:]),
                            start=(et == 0), stop=(et == NE - 1),
                        )
                    nc.scalar.activation(
                        kt_tiles[dt][:, ts(kb, NB)], ps[:], AF.Identity, bias=bk_t[dt][:]
                    )

            # ---- phase Q+S fused: per q-block project Q, then scores+exp ----
            pt_tiles = []
            rs_t = []
            for kt in range(NKT):
                ptt = pt_pool.tile([P, S], bf16, tag=f"pt{kt}", name=f"ptt{kt}")
                pt_tiles.append(ptt)
                rst = small_pool.tile([P, NQB], f32, tag=f"rs{kt}", name=f"rst{kt}")
                rs_t.append(rst)
            wq_t = {}
            for qb in range(NQB):
                xq = []
                for et in range(NE):
                    t = x_pool.tile([P, NB], dt_in, tag="x", name=f"xq{qb}_{et}")
                    nc.sync.dma_start(t[:], xt_d[ts(et, P), ts(qb, NB)])
                    xq.append(t)
                qt_t = []
                for dt in range(ND):
                    if qb == 0:
                        for et in range(NE):
                            w = w_pool.tile([P, P], dt_in, tag="w", name=f"wq{et}_{dt}")
                            nc.sync.dma_start(w[:], wq_d[ts(et, P), ts(dt, P)])
                            wq_t[(et, dt)] = w
                    ps = ps_pool.tile([P, NB], f32, tag="ps", name="psq")
                    for et in range(NE):
                        nc.tensor.matmul(
                            ps[:], r(wq_t[(et, dt)][:]), r(xq[et][:]),
                            start=(et == 0), stop=(et == NE - 1),
                        )
                    q = qt_pool.tile([P, NB], dt_in, tag="qt", name=f"qtt{qb}_{dt}")
                    nc.scalar.activation(q[:], ps[:], AF.Identity, bias=bq_t[dt][:])
                    qt_t.append(q)
                for kt in range(NKT):
                    ps = ps_pool.tile([P, NB], f32, tag="ps", name="pss")
                    for dt in range(ND):
                        nc.tensor.matmul(
                            ps[:], r(kt_tiles[dt][:, ts(kt, P)]), r(qt_t[dt][:]),
                            start=(dt == 0), stop=(dt == ND - 1),
                        )
                    # PT = exp(scores/sqrt(D)); row-sum (over q) accumulated
                    nc.scalar.activation(
                        pt_tiles[kt][:, ts(qb, NB)], ps[:], AF.Exp,
                        scale=inv_sqrt_d,
                        accum_out=rs_t[kt][:, qb:qb + 1],
                    )

            # softmax denominators 1/D[k] (fully local: full q range on-core)
            rcp_t = []
            for kt in range(NKT):
                rsum = small_pool.tile([P, 1], f32, tag=f"rsum{kt}", name=f"rsum{kt}")
                nc.vector.reduce_sum(rsum[:], rs_t[kt][:], axis=X)
                rcp = small_pool.tile([P, 1], f32, tag=f"rcp{kt}", name=f"rcp{kt}")
                nc.vector.reciprocal(rcp[:], rsum[:])
                rcp_t.append(rcp)

            # ---- phase V: V[k, dv] = (X@Wv + bv) * (1/D[k]) ----
            v_tiles = []
            for kt in range(NKT):
                vt = v_pool.tile([P, D], bf16, tag=f"v{kt}", name=f"vt{kt}")
                v_tiles.append(vt)
            KT_PER_B = NB // P  # k-tiles per 512-block
            wv_t = {}
            for kh in range(NKB):
                xv = []
                for et in range(NE):
                    t = x_pool.tile([P, NB], dt_in, tag="x", name=f"xv{kh}_{et}")
                    nc.sync.dma_start(t[:], xth_d[ts(et, P), ts(kh, NB)])
                    xv.append(t)
                for dvb in range(NDVB):
                    if kh == 0:
                        for et in range(NE):
                            w = w_pool.tile([P, NB], dt_in, tag="wv", bufs=2 * NE,
                                            name=f"wv{et}_{dvb}")
                            nc.sync.dma_start(w[:], wv_d[ts(et, P), ts(dvb, NB)])
                            wv_t[(et, dvb)] = w
                    for kt4 in range(KT_PER_B):
                        kt = kh * KT_PER_B + kt4
                        ps = ps_pool.tile([P, NB], f32, tag="ps", name="psv")
                        for et in range(NE):
                            nc.tensor.matmul(
                                ps[:], r(xv[et][:, ts(kt4, P)]),
                                r(wv_t[(et, dvb)][:]),
                                start=(et == 0), stop=False,
                            )
                        # += 1^T @ bv  (broadcasts bv along k rows)
                        nc.tensor.matmul(
                            ps[:], r(ones_t[:]), r(bv_t[:, ts(dvb, NB)]),
                            start=False, stop=True,
                        )
                        nc.vector.tensor_scalar_mul(
                            v_tiles[kt][:, ts(dvb, NB)], ps[:], rcp_t[kt][:]
                        )

            # ---- phase AV: part[q, dv] = sum_{k in my half} PT[k,q] * V[k,dv],
            # with the pairwise ReduceScatter chunked to overlap AV compute ----
            NCH = cfg.NCH
            QT_PER_CH = NQT // NCH
            CH_ROWS = QT_PER_CH * P           # part rows per chunk
            CH_OUT = CH_ROWS // 2             # rsout/y rows per chunk
            for ch in range(NCH):
                for qt in range(ch * QT_PER_CH, (ch + 1) * QT_PER_CH):
                    for dvb in range(NDVB):
                        ps = ps_pool.tile([P, NB], f32, tag="ps", name="psav")
                        for kt in range(NKT):
                            nc.tensor.matmul(
                                ps[:], pt_tiles[kt][:, ts(qt, P)],
                                v_tiles[kt][:, ts(dvb, NB)],
                                start=(kt == 0), stop=(kt == NKT - 1),
                            )
                        ob = out_pool.tile([P, NB], f32, tag="ob", name="ob")
                        nc.scalar.copy(ob[:], ps[:])
                        nc.sync.dma_start(part_d[ts(qt, P), ts(dvb, NB)], ob[:])
                nc.gpsimd.collective_compute(
                    "ReduceScatter",
                    mybir.AluOpType.add,
                    replica_groups=cfg.groups,
                    ins=[part_d[ts(ch, CH_ROWS), :].opt()],
                    outs=[rsout_d[ts(ch, CH_OUT), :].opt()],
                )
                nc.sync.dma_start(y_d[ts(ch, CH_OUT), :], rsout_d[ts(ch, CH_OUT), :])

    nc.compile()
    return nc


def make_in_maps(cfg: Cfg, x, Wq, bq, Wk, bk, Wv, bv):
    SH = cfg.SH
    f32 = np.float32
    if cfg.mm == "bf16":
        import ml_dtypes
        dt_in = ml_dtypes.bfloat16
    else:
        dt_in = f32
    in_maps = []
    shared = {
        "wq": np.ascontiguousarray(Wq, dtype=dt_in),
        "wk": np.ascontiguousarray(Wk, dtype=dt_in),
        "wv": np.ascontiguousarray(Wv, dtype=dt_in),
        "bq": np.ascontiguousarray(np.reshape(bq, (-1, 1)), dtype=f32),
        "bk": np.ascontiguousarray(np.reshape(bk, (-1, 1)), dtype=f32),
        "bv": np.ascontiguousarray(np.reshape(bv, (1, -1)), dtype=dt_in),
        "onesv": np.ones((1, 128), dtype=dt_in),
    }
    for c in range(cfg.n_cores):
        b, h = c // 2, c % 2
        xb = np.asarray(x[b], dtype=f32)
        m = dict(shared)
        m["xt"] = np.ascontiguousarray(xb.T, dtype=dt_in)
        m["xth"] = np.ascontiguousarray(xb[h * SH:(h + 1) * SH, :].T, dtype=dt_in)
        in_maps.append(m)
    return in_maps


def run(inputs: dict, cfg: Cfg = PROD, trace: bool = False):
    from concourse.bass_utils import run_bass_kernel_spmd

    nc = build_nc(cfg)
    in_maps = make_in_maps(cfg, inputs["x"], inputs["Wq"], inputs["bq"],
                           inputs["Wk"], inputs["bk"], inputs["Wv"], inputs["bv"])
    res = run_bass_kernel_spmd(nc, in_maps, list(range(cfg.n_cores)), trace=trace)
    out = assemble(cfg, [r["y"] for r in res.results])
    return out, res


def assemble(cfg: Cfg, ys):
    """Each RS chunk (CH q-rows of a batch) is scattered between the pair:
    rank r gets the chunk's rows [r*CH/2, (r+1)*CH/2)."""
    B, S, D = cfg.B, cfg.S, cfg.D
    CH = S // cfg.NCH      # part rows per chunk
    H = CH // 2            # rows each rank holds per chunk
    out = np.empty((B, S, D), dtype=np.float32)
    for b in range(B):
        for ch in range(cfg.NCH):
            out[b, ch * CH: ch * CH + H] = ys[2 * b][ch * H:(ch + 1) * H]
            out[b, ch * CH + H:(ch + 1) * CH] = ys[2 * b + 1][ch * H:(ch + 1) * H]
    return out


def kernel(**inputs) -> np.ndarray:
    out, _ = run(inputs, PROD, trace=False)
    return out


# revision 20
# speedup vs baseline: 1.1567x; 1.1567x over previous
"""Multi-head-free attention (softmax over the QUERY axis) on 8 trn2 NeuronCores.

Problem: x:[4,2048,1024], Wq/Wk/Wv:[1024,1024], bq/bk/bv:[1024]
    q = x@Wq+bq ; k = x@Wk+bk ; v = x@Wv+bv
    scores = einsum('bqd,bkd->bqk', q, k) / 32
    attn   = softmax(scores, axis=1)          # over q (dim 1)!
    out    = einsum('bqk,bkv->bqv', attn, v)

Sharding: 4 batches x 2-way split of the QUERY axis across 8 cores
(core c -> batch c//2, query-half c%2).  Each core computes its q-rows
of the output against the FULL key range, so the only cross-core
dependency is the softmax denominator D[k] = sum_q exp(s[q,k]) -- an
8 KB pairwise AllReduce that hides behind the V-projection phase.
There is no output collective: each core DMAs its q-rows directly.

All matmuls run as float32r (full PE rate at N=512 moving dim, fp32
storage).  The attn*V contraction runs in bf16 (attn weights + V), with
fp32 PSUM accumulation.
"""

import sys

if "/opt/trn_rl_repo" not in sys.path:
    sys.path.insert(0, "/opt/trn_rl_repo")

import numpy as np

P = 128  # SBUF partitions


class Cfg:
    def __init__(self, B=4, S=2048, E=1024, D=1024, NB=512, n_cores=8, mm="f32r"):
        self.B, self.S, self.E, self.D, self.NB = B, S, E, D, NB
        self.mm = mm
        self.SH = S // 2          # per-core query-half length
        self.NE = E // P          # e (contraction) tiles
        self.ND = D // P          # d tiles
        self.NQb = self.SH // NB  # q 512-blocks (my half)
        self.NKb = S // NB        # k 512-blocks (full)
        self.NKt = S // P         # k 128-tiles (full)
        self.NQt = self.SH // P   # q 128-tiles (my half)
        self.NDVB = D // NB       # dv 512-blocks
        self.n_cores = n_cores
        self.groups = [[2 * i, 2 * i + 1] for i in range(n_cores // 2)]


PROD = Cfg()


def build_nc(cfg: Cfg):
    from concourse import bacc, bass, mybir, tile

    f32 = mybir.dt.float32
    f32r = mybir.dt.float32r
    bf16 = mybir.dt.bfloat16
    AF = mybir.ActivationFunctionType
    X = mybir.AxisListType.X
    ts = bass.ts

    B, S, E, D, NB = cfg.B, cfg.S, cfg.E, cfg.D, cfg.NB
    SH, NE, ND = cfg.SH, cfg.NE, cfg.ND
    NQb, NKb, NKt, NQt, NDVB = cfg.NQb, cfg.NKb, cfg.NKt, cfg.NQt, cfg.NDVB
    KT_PER_B = NB // P
    inv_sqrt_d = 1.0 / float(np.sqrt(np.float32(D)))

    nc = bacc.Bacc(None, num_devices=cfg.n_cores)
    dt_in = bf16 if cfg.mm == "bf16" else f32r

    # Per-core inputs (host pre-shards / pre-transposes).
    # xt: X^T for the whole batch (K and V read all tokens).
    # xth: X^T columns of MY query-half (Q reads only those).
    xt_d = nc.declare_dram_parameter("xt", [E, S], dt_in, isOutput=False)
    xth_d = nc.declare_dram_parameter("xth", [E, SH], dt_in, isOutput=False)
    wq_d = nc.declare_dram_parameter("wq", [E, D], dt_in, isOutput=False)
    wk_d = nc.declare_dram_parameter("wk", [E, D], dt_in, isOutput=False)
    wv_d = nc.declare_dram_parameter("wv", [E, D], dt_in, isOutput=False)
    bq_d = nc.declare_dram_parameter("bq", [D, 1], f32, isOutput=False)
    bk_d = nc.declare_dram_parameter("bk", [D, 1], f32, isOutput=False)
    bv_d = nc.declare_dram_parameter("bv", [1, D], dt_in, isOutput=False)
    ones_d = nc.declare_dram_parameter("onesv", [1, P], dt_in, isOutput=False)
    y_d = nc.declare_dram_parameter("y", [SH, D], f32, isOutput=True)

    def r(ap):  # tensors feeding the PE are already dt_in (f32r or bf16)
        return ap

    with tile.TileContext(nc) as tc:
        with (
            tc.tile_pool(name="w", bufs=8 * ND + 2) as w_pool,
            tc.tile_pool(name="x", bufs=9) as x_pool,
            tc.tile_pool(name="qt", bufs=1) as qt_pool,
            tc.tile_pool(name="ktw", bufs=11) as ktw_pool,
            tc.tile_pool(name="pt", bufs=1) as pt_pool,
            tc.tile_pool(name="v", bufs=1) as v_pool,
            tc.tile_pool(name="wv", bufs=2 + NE) as wv_pool,
            tc.tile_pool(name="ob", bufs=3) as out_pool,
            tc.tile_pool(name="small", bufs=1) as small_pool,
            tc.tile_pool(name="ps", bufs=8, space="PSUM") as ps_pool,
            tc.tile_pool(name="dram", bufs=1, space="DRAM") as dram_pool,
        ):
            ar_in = dram_pool.tile([P, NKt], f32)
            ar_out = dram_pool.tile([P, NKt], f32)

            # ---- constants ----
            ones_t = small_pool.tile([1, P], dt_in, tag="ones")
            nc.sync.dma_start(ones_t[:], ones_d[:])
            bv_t = small_pool.tile([1, D], dt_in, tag="bvrow")
            nc.sync.dma_start(bv_t[:], bv_d[:])
            bq_t, bk_t = [], []
            for dt in range(ND):
                bqt = small_pool.tile([P, 1], f32, tag=f"bq{dt}")
                nc.sync.dma_start(bqt[:], bq_d[ts(dt, P), :])
                bq_t.append(bqt)
                bkt = small_pool.tile([P, 1], f32, tag=f"bk{dt}")
                nc.sync.dma_start(bkt[:], bk_d[ts(dt, P), :])
                bk_t.append(bkt)

            # ---- phase Q: QT[dt][d, q] = Q[q, d]^T for MY q-half (resident) ----
            qt_tiles = []
            for dt in range(ND):
                q = qt_pool.tile([P, SH], dt_in, tag=f"qt{dt}", name=f"qtt{dt}")
                qt_tiles.append(q)
            wq_t = {}
            for qb in range(NQb):
                xq = []
                for et in range(NE):
                    t = x_pool.tile([P, NB], dt_in, tag="x", name=f"xq{qb}_{et}")
                    nc.sync.dma_start(t[:], xth_d[ts(et, P), ts(qb, NB)])
                    xq.append(t)
                for dt in range(ND):
                    if qb == 0:
                        for et in range(NE):
                            w = w_pool.tile([P, P], dt_in, tag="w", name=f"wq{et}_{dt}")
                            nc.sync.dma_start(w[:], wq_d[ts(et, P), ts(dt, P)])
                            wq_t[(et, dt)] = w
                    ps = ps_pool.tile([P, NB], f32, tag="ps", name="psq")
                    for et in range(NE):
                        nc.tensor.matmul(
                            ps[:], r(wq_t[(et, dt)][:]), r(xq[et][:]),
                            start=(et == 0), stop=(et == NE - 1),
                        )
                    nc.scalar.activation(
                        qt_tiles[dt][:, ts(qb, NB)], ps[:], AF.Identity, bias=bq_t[dt][:]
                    )

            # ---- phase K+S fused: per 512-token k-block, project K then score ----
            # PT[kt][k, q] = exp(scores[q, k] / sqrt(D)) for all k, my q-half
            pt_tiles = []
            rs_t = []
            for kt in range(NKt):
                ptt = pt_pool.tile([P, SH], bf16, tag=f"pt{kt}", name=f"ptt{kt}")
                pt_tiles.append(ptt)
                rst = small_pool.tile([P, NQb], f32, tag=f"rs{kt}", name=f"rst{kt}")
                rs_t.append(rst)
            rs_all = small_pool.tile([P, NKt], f32, tag="rsall")
            wk_t = {}
            for kb in range(NKb):
                xk = []
                for et in range(NE):
                    t = x_pool.tile([P, NB], dt_in, tag="x", name=f"xk{kb}_{et}")
                    nc.sync.dma_start(t[:], xt_d[ts(et, P), ts(kb, NB)])
                    xk.append(t)
                ktw = []
                for dt in range(ND):
                    if kb == 0:
                        for et in range(NE):
                            w = w_pool.tile([P, P], dt_in, tag="w", name=f"wk{et}_{dt}")
                            nc.sync.dma_start(w[:], wk_d[ts(et, P), ts(dt, P)])
                            wk_t[(et, dt)] = w
                    ps = ps_pool.tile([P, NB], f32, tag="ps", name="psk")
                    for et in range(NE):
                        nc.tensor.matmul(
                            ps[:], r(wk_t[(et, dt)][:]), r(xk[et][:]),
                            start=(et == 0), stop=(et == NE - 1),
                        )
                    kw = ktw_pool.tile([P, NB], dt_in, tag="ktw", name=f"ktw{kb}_{dt}")
                    nc.scalar.activation(kw[:], ps[:], AF.Identity, bias=bk_t[dt][:])
                    ktw.append(kw)
                for kt4 in range(KT_PER_B):
                    kt = kb * KT_PER_B + kt4
                    for qb in range(NQb):
                        ps = ps_pool.tile([P, NB], f32, tag="ps", name="pss")
                        for dt in range(ND):
                            nc.tensor.matmul(
                                ps[:], r(ktw[dt][:, ts(kt4, P)]),
                                r(qt_tiles[dt][:, ts(qb, NB)]),
                                start=(dt == 0), stop=(dt == ND - 1),
                            )
                        nc.scalar.activation(
                            pt_tiles[kt][:, ts(qb, NB)], ps[:], AF.Exp,
                            scale=inv_sqrt_d,
                            accum_out=rs_t[kt][:, qb:qb + 1],
                        )
                    nc.vector.reduce_sum(rs_all[:, kt:kt + 1], rs_t[kt][:], axis=X)

            # ---- softmax denominators: pairwise 8KB AllReduce, then 1/D ----
            nc.sync.dma_start(ar_in[:], rs_all[:])
            nc.gpsimd.collective_compute(
                "AllReduce",
                mybir.AluOpType.add,
                replica_groups=cfg.groups,
                ins=[ar_in[:].opt()],
                outs=[ar_out[:].opt()],
            )
            rsum_t = small_pool.tile([P, NKt], f32, tag="rsum")
            nc.sync.dma_start(rsum_t[:], ar_out[:])
            rcp_all = small_pool.tile([P, NKt], f32, tag="rcp")
            nc.vector.reciprocal(rcp_all[:], rsum_t[:])

            # ---- phase V: V[kt][k, dv] = X@Wv + bv (full tokens; no rcp dep) ----
            v_tiles = []
            for kt in range(NKt):
                vt = v_pool.tile([P, D], bf16, tag=f"v{kt}", name=f"vt{kt}")
                v_tiles.append(vt)
            wv_t = {}
            for dvb in range(NDVB):
                for et in range(NE):
                    w = wv_pool.tile([P, NB], dt_in, tag="wv", name=f"wv{et}_{dvb}")
                    nc.sync.dma_start(w[:], wv_d[ts(et, P), ts(dvb, NB)])
                    wv_t[(et, dvb)] = w
                for kb in range(NKb):
                    xv = []
                    for et in range(NE):
                        t = x_pool.tile([P, NB], dt_in, tag="x", name=f"xv{dvb}_{kb}_{et}")
                        nc.sync.dma_start(t[:], xt_d[ts(et, P), ts(kb, NB)])
                        xv.append(t)
                    for kt4 in range(KT_PER_B):
                        kt = kb * KT_PER_B + kt4
                        ps = ps_pool.tile([P, NB], f32, tag="ps", name="psv")
                        for et in range(NE):
                            nc.tensor.matmul(
                                ps[:], r(xv[et][:, ts(kt4, P)]),
                                r(wv_t[(et, dvb)][:]),
                                start=(et == 0), stop=False,
                            )
                        # += 1^T @ bv  (broadcasts bv along k rows)
                        nc.tensor.matmul(
                            ps[:], r(ones_t[:]), r(bv_t[:, ts(dvb, NB)]),
                            start=False, stop=True,
                        )
                        nc.vector.tensor_copy(v_tiles[kt][:, ts(dvb, NB)], ps[:])

            # attn = PT * (1/D[k]) -- per-partition (k) scale, in place, on DVE
            for kt in range(NKt):
                nc.vector.tensor_scalar_mul(
                    pt_tiles[kt][:], pt_tiles[kt][:], rcp_all[:, kt:kt + 1]
                )

            # ---- phase AV: y[q, dv] = sum_k attn[k,q] * V[k,dv]; direct DMA out ----
            for qt in range(NQt):
                for dvb in range(NDVB):
                    ps = ps_pool.tile([P, NB], f32, tag="ps", name="psav")
                    for kt in range(NKt):
                        nc.tensor.matmul(
                            ps[:], pt_tiles[kt][:, ts(qt, P)],
                            v_tiles[kt][:, ts(dvb, NB)],
                            start=(kt == 0), stop=(kt == NKt - 1),
                        )
                    ob = out_pool.tile([P, NB], f32, tag="ob", name="ob")
                    nc.scalar.copy(ob[:], ps[:])
                    nc.sync.dma_start(y_d[ts(qt, P), ts(dvb, NB)], ob[:])

    nc.compile()
    return nc


def make_in_maps(cfg: Cfg, x, Wq, bq, Wk, bk, Wv, bv):
    SH = cfg.SH
    f32 = np.float32
    if cfg.mm == "bf16":
        import ml_dtypes
        dt_in = ml_dtypes.bfloat16
    else:
        dt_in = f32
    in_maps = []
    shared = {
        "wq": np.ascontiguousarray(Wq, dtype=dt_in),
        "wk": np.ascontiguousarray(Wk, dtype=dt_in),
        "wv": np.ascontiguousarray(Wv, dtype=dt_in),
        "bq": np.ascontiguousarray(np.reshape(bq, (-1, 1)), dtype=f32),
        "bk": np.ascontiguousarray(np.reshape(bk, (-1, 1)), dtype=f32),
        "bv": np.ascontiguousarray(np.reshape(bv, (1, -1)), dtype=dt_in),
        "onesv": np.ones((1, 128), dtype=dt_in),
    }
    for c in range(cfg.n_cores):
        b, h = c // 2, c % 2
        xb = np.asarray(x[b], dtype=f32)
        m = dict(shared)
        m["xt"] = np.ascontiguousarray(xb.T, dtype=dt_in)
        m["xth"] = np.ascontiguousarray(xb[h * SH:(h + 1) * SH, :].T, dtype=dt_in)
        in_maps.append(m)
    return in_maps


def run(inputs: dict, cfg: Cfg = PROD, trace: bool = False):
    from concourse.bass_utils import run_bass_kernel_spmd

    nc = build_nc(cfg)
    in_maps = make_in_maps(cfg, inputs["x"], inputs["Wq"], inputs["bq"],
                           inputs["Wk"], inputs["bk"], inputs["Wv"], inputs["bv"])
    res = run_bass_kernel_spmd(nc, in_maps, list(range(cfg.n_cores)), trace=trace)
    out = assemble(cfg, [r["y"] for r in res.results])
    return out, res


def assemble(cfg: Cfg, ys):
    """Core 2b holds q-rows [0, S/2), core 2b+1 holds [S/2, S) of batch b."""
    B, S, D = cfg.B, cfg.S, cfg.D
    out = np.empty((B, S, D), dtype=np.float32)
    for b in range(B):
        out[b, : cfg.SH] = ys[2 * b]
        out[b, cfg.SH:] = ys[2 * b + 1]
    return out


def kernel(**inputs) -> np.ndarray:
    out, _ = run(inputs, PROD, trace=False)
    return out


# revision 23
# speedup vs baseline: 1.2073x; 1.0437x over previous
"""Multi-head-free attention (softmax over the QUERY axis) on 8 trn2 NeuronCores.

Problem: x:[4,2048,1024], Wq/Wk/Wv:[1024,1024], bq/bk/bv:[1024]
    q = x@Wq+bq ; k = x@Wk+bk ; v = x@Wv+bv
    scores = einsum('bqd,bkd->bqk', q, k) / 32
    attn   = softmax(scores, axis=1)          # over q (dim 1)!
    out    = einsum('bqk,bkv->bqv', attn, v)

Sharding: 4 batches x 2-way split of the QUERY axis across 8 cores
(core c -> batch c//2, query-half c%2).  Each core computes its q-rows
of the output against the FULL key range, so the only cross-core
dependency is the softmax denominator D[k] = sum_q exp(s[q,k]) -- an
8 KB pairwise AllReduce that hides behind the V-projection phase.
There is no output collective: each core DMAs its q-rows directly.

All matmuls run as float32r (full PE rate at N=512 moving dim, fp32
storage).  The attn*V contraction runs in bf16 (attn weights + V), with
fp32 PSUM accumulation.
"""

import sys

if "/opt/trn_rl_repo" not in sys.path:
    sys.path.insert(0, "/opt/trn_rl_repo")

import numpy as np

P = 128  # SBUF partitions


class Cfg:
    def __init__(self, B=4, S=2048, E=1024, D=1024, NB=512, n_cores=8, mm="f32r"):
        self.B, self.S, self.E, self.D, self.NB = B, S, E, D, NB
        self.mm = mm
        self.SH = S // 2          # per-core query-half length
        self.NE = E // P          # e (contraction) tiles
        self.ND = D // P          # d tiles
        self.NQb = self.SH // NB  # q 512-blocks (my half)
        self.NKb = S // NB        # k 512-blocks (full)
        self.NKt = S // P         # k 128-tiles (full)
        self.NQt = self.SH // P   # q 128-tiles (my half)
        self.NDVB = D // NB       # dv 512-blocks
        self.n_cores = n_cores
        self.groups = [[2 * i, 2 * i + 1] for i in range(n_cores // 2)]


PROD = Cfg()


def build_nc(cfg: Cfg):
    from concourse import bacc, bass, mybir, tile

    f32 = mybir.dt.float32
    f32r = mybir.dt.float32r
    bf16 = mybir.dt.bfloat16
    AF = mybir.ActivationFunctionType
    X = mybir.AxisListType.X
    ts = bass.ts

    B, S, E, D, NB = cfg.B, cfg.S, cfg.E, cfg.D, cfg.NB
    SH, NE, ND = cfg.SH, cfg.NE, cfg.ND
    NQb, NKb, NKt, NQt, NDVB = cfg.NQb, cfg.NKb, cfg.NKt, cfg.NQt, cfg.NDVB
    KT_PER_B = NB // P
    inv_sqrt_d = 1.0 / float(np.sqrt(np.float32(D)))

    nc = bacc.Bacc(None, num_devices=cfg.n_cores)
    dt_in = bf16 if cfg.mm == "bf16" else f32r

    # Per-core inputs (host pre-shards / pre-transposes).
    # xt: X^T for the whole batch (K and V read all tokens).
    # xth: X^T columns of MY query-half (Q reads only those).
    xt_d = nc.declare_dram_parameter("xt", [E, S], dt_in, isOutput=False)
    xth_d = nc.declare_dram_parameter("xth", [E, SH], dt_in, isOutput=False)
    wq_d = nc.declare_dram_parameter("wq", [E, D], dt_in, isOutput=False)
    wk_d = nc.declare_dram_parameter("wk", [E, D], dt_in, isOutput=False)
    wv_d = nc.declare_dram_parameter("wv", [E, D], dt_in, isOutput=False)
    bq_d = nc.declare_dram_parameter("bq", [D, 1], f32, isOutput=False)
    bk_d = nc.declare_dram_parameter("bk", [D, 1], f32, isOutput=False)
    bvb_d = nc.declare_dram_parameter("bvb", [P, D], f32, isOutput=False)
    y_d = nc.declare_dram_parameter("y", [SH, D], f32, isOutput=True)

    def r(ap):  # tensors feeding the PE are already dt_in (f32r or bf16)
        return ap

    with tile.TileContext(nc) as tc:
        with (
            tc.tile_pool(name="w", bufs=8 * ND + 1) as w_pool,
            tc.tile_pool(name="x", bufs=11) as x_pool,
            tc.tile_pool(name="qt", bufs=1) as qt_pool,
            tc.tile_pool(name="ktw", bufs=10) as ktw_pool,
            tc.tile_pool(name="pt", bufs=1) as pt_pool,
            tc.tile_pool(name="v", bufs=1) as v_pool,
            tc.tile_pool(name="wv", bufs=NE) as wv_pool,
            tc.tile_pool(name="ob", bufs=3) as out_pool,
            tc.tile_pool(name="small", bufs=1) as small_pool,
            tc.tile_pool(name="ps", bufs=8, space="PSUM") as ps_pool,
            tc.tile_pool(name="dram", bufs=1, space="DRAM") as dram_pool,
        ):
            ar_in = dram_pool.tile([P, NKt], f32)
            ar_out = dram_pool.tile([P, NKt], f32)

            # ---- constants ----
            bvb_t = small_pool.tile([P, D], f32, tag="bvb")
            nc.sync.dma_start(bvb_t[:], bvb_d[:])
            bq_t, bk_t = [], []
            for dt in range(ND):
                bqt = small_pool.tile([P, 1], f32, tag=f"bq{dt}")
                nc.sync.dma_start(bqt[:], bq_d[ts(dt, P), :])
                bq_t.append(bqt)
                bkt = small_pool.tile([P, 1], f32, tag=f"bk{dt}")
                nc.sync.dma_start(bkt[:], bk_d[ts(dt, P), :])
                bk_t.append(bkt)

            # ---- phase Q: QT[dt][d, q] = Q[q, d]^T for MY q-half (resident) ----
            qt_tiles = []
            for dt in range(ND):
                q = qt_pool.tile([P, SH], dt_in, tag=f"qt{dt}", name=f"qtt{dt}")
                qt_tiles.append(q)
            wq_t = {}
            for qb in range(NQb):
                xq = []
                for et in range(NE):
                    t = x_pool.tile([P, NB], dt_in, tag="x", name=f"xq{qb}_{et}")
                    nc.sync.dma_start(t[:], xth_d[ts(et, P), ts(qb, NB)])
                    xq.append(t)
                for dt in range(ND):
                    if qb == 0:
                        for et in range(NE):
                            w = w_pool.tile([P, P], dt_in, tag="w", name=f"wq{et}_{dt}")
                            nc.sync.dma_start(w[:], wq_d[ts(et, P), ts(dt, P)])
                            wq_t[(et, dt)] = w
                    ps = ps_pool.tile([P, NB], f32, tag="ps", name="psq")
                    for et in range(NE):
                        nc.tensor.matmul(
                            ps[:], r(wq_t[(et, dt)][:]), r(xq[et][:]),
                            start=(et == 0), stop=(et == NE - 1),
                        )
                    nc.scalar.activation(
                        qt_tiles[dt][:, ts(qb, NB)], ps[:], AF.Identity, bias=bq_t[dt][:]
                    )

            # prefetch Wv's first dv-half early (bandwidth is idle during K+S)
            wv_t = {}
            for et in range(NE):
                w = wv_pool.tile([P, NB], dt_in, tag="wv", name=f"wv{et}_0")
                nc.sync.dma_start(w[:], wv_d[ts(et, P), ts(0, NB)])
                wv_t[(et, 0)] = w

            # ---- phase K+S fused: per 512-token k-block, project K then score ----
            # PT[kt][k, q] = exp(scores[q, k] / sqrt(D)) for all k, my q-half
            pt_tiles = []
            rs_t = []
            for kt in range(NKt):
                ptt = pt_pool.tile([P, SH], bf16, tag=f"pt{kt}", name=f"ptt{kt}")
                pt_tiles.append(ptt)
                rst = small_pool.tile([P, NQb], f32, tag=f"rs{kt}", name=f"rst{kt}")
                rs_t.append(rst)
            rs_all = small_pool.tile([P, NKt], f32, tag="rsall")
            wk_t = {}
            for kb in range(NKb):
                xk = []
                for et in range(NE):
                    t = x_pool.tile([P, NB], dt_in, tag="x", name=f"xk{kb}_{et}")
                    nc.sync.dma_start(t[:], xt_d[ts(et, P), ts(kb, NB)])
                    xk.append(t)
                ktw = []
                for dt in range(ND):
                    if kb == 0:
                        for et in range(NE):
                            w = w_pool.tile([P, P], dt_in, tag="w", name=f"wk{et}_{dt}")
                            nc.sync.dma_start(w[:], wk_d[ts(et, P), ts(dt, P)])
                            wk_t[(et, dt)] = w
                    ps = ps_pool.tile([P, NB], f32, tag="ps", name="psk")
                    for et in range(NE):
                        nc.tensor.matmul(
                            ps[:], r(wk_t[(et, dt)][:]), r(xk[et][:]),
                            start=(et == 0), stop=(et == NE - 1),
                        )
                    kw = ktw_pool.tile([P, NB], dt_in, tag="ktw", name=f"ktw{kb}_{dt}")
                    nc.scalar.activation(kw[:], ps[:], AF.Identity, bias=bk_t[dt][:])
                    ktw.append(kw)
                for kt4 in range(KT_PER_B):
                    kt = kb * KT_PER_B + kt4
                    for qb in range(NQb):
                        ps = ps_pool.tile([P, NB], f32, tag="ps", name="pss")
                        for dt in range(ND):
                            nc.tensor.matmul(
                                ps[:], r(ktw[dt][:, ts(kt4, P)]),
                                r(qt_tiles[dt][:, ts(qb, NB)]),
                                start=(dt == 0), stop=(dt == ND - 1),
                            )
                        nc.scalar.activation(
                            pt_tiles[kt][:, ts(qb, NB)], ps[:], AF.Exp,
                            scale=inv_sqrt_d,
                            accum_out=rs_t[kt][:, qb:qb + 1],
                        )
                    nc.vector.reduce_sum(rs_all[:, kt:kt + 1], rs_t[kt][:], axis=X)

            # ---- softmax denominators: pairwise 8KB AllReduce, then 1/D ----
            nc.sync.dma_start(ar_in[:], rs_all[:])
            nc.gpsimd.collective_compute(
                "AllReduce",
                mybir.AluOpType.add,
                replica_groups=cfg.groups,
                ins=[ar_in[:].opt()],
                outs=[ar_out[:].opt()],
            )
            rsum_t = small_pool.tile([P, NKt], f32, tag="rsum")
            nc.sync.dma_start(rsum_t[:], ar_out[:])
            rcp_all = small_pool.tile([P, NKt], f32, tag="rcp")
            nc.vector.reciprocal(rcp_all[:], rsum_t[:])

            # ---- phase V: V[kt][k, dv] = X@Wv + bv (full tokens; no rcp dep) ----
            v_tiles = []
            for kt in range(NKt):
                vt = v_pool.tile([P, D], bf16, tag=f"v{kt}", name=f"vt{kt}")
                v_tiles.append(vt)
            # Wv's second dv-half (if any) reuses QT slots (QT dead after K+S)
            for dvb in range(1, NDVB):
                for et in range(NE):
                    w = qt_pool.tile([P, NB], dt_in, tag=f"qt{et}", name=f"wv{et}_{dvb}")
                    nc.sync.dma_start(w[:], wv_d[ts(et, P), ts(dvb, NB)])
                    wv_t[(et, dvb)] = w
            for kb in range(NKb):
                xv = []
                for et in range(NE):
                    t = x_pool.tile([P, NB], dt_in, tag="x", name=f"xv{kb}_{et}")
                    nc.sync.dma_start(t[:], xt_d[ts(et, P), ts(kb, NB)])
                    xv.append(t)
                for kt4 in range(KT_PER_B):
                    kt = kb * KT_PER_B + kt4
                    for dvb in range(NDVB):
                        ps = ps_pool.tile([P, NB], f32, tag="ps", name="psv")
                        for et in range(NE):
                            nc.tensor.matmul(
                                ps[:], r(xv[et][:, ts(kt4, P)]),
                                r(wv_t[(et, dvb)][:]),
                                start=(et == 0), stop=(et == NE - 1),
                            )
                        nc.vector.tensor_add(
                            v_tiles[kt][:, ts(dvb, NB)], ps[:], bvb_t[:, ts(dvb, NB)]
                        )

            # attn = PT * (1/D[k]) -- per-partition (k) scale, in place, on DVE
            for kt in range(NKt):
                nc.vector.tensor_scalar_mul(
                    pt_tiles[kt][:], pt_tiles[kt][:], rcp_all[:, kt:kt + 1]
                )

            # ---- phase AV: y[q, dv] = sum_k attn[k,q] * V[k,dv]; direct DMA out ----
            for qt in range(NQt):
                for dvb in range(NDVB):
                    ps = ps_pool.tile([P, NB], f32, tag="ps", name="psav")
                    for kt in range(NKt):
                        nc.tensor.matmul(
                            ps[:], pt_tiles[kt][:, ts(qt, P)],
                            v_tiles[kt][:, ts(dvb, NB)],
                            start=(kt == 0), stop=(kt == NKt - 1),
                        )
                    ob = out_pool.tile([P, NB], f32, tag="ob", name="ob")
                    nc.scalar.copy(ob[:], ps[:])
                    nc.sync.dma_start(y_d[ts(qt, P), ts(dvb, NB)], ob[:])

    nc.compile()
    return nc


def make_in_maps(cfg: Cfg, x, Wq, bq, Wk, bk, Wv, bv):
    SH = cfg.SH
    f32 = np.float32
    if cfg.mm == "bf16":
        import ml_dtypes
        dt_in = ml_dtypes.bfloat16
    else:
        dt_in = f32
    in_maps = []
    shared = {
        "wq": np.ascontiguousarray(Wq, dtype=dt_in),
        "wk": np.ascontiguousarray(Wk, dtype=dt_in),
        "wv": np.ascontiguousarray(Wv, dtype=dt_in),
        "bq": np.ascontiguousarray(np.reshape(bq, (-1, 1)), dtype=f32),
        "bk": np.ascontiguousarray(np.reshape(bk, (-1, 1)), dtype=f32),
        "bvb": np.ascontiguousarray(np.broadcast_to(np.reshape(bv, (1, -1)), (128, len(np.ravel(bv)))), dtype=f32),
    }
    for c in range(cfg.n_cores):
        b, h = c // 2, c % 2
        xb = np.asarray(x[b], dtype=f32)
        m = dict(shared)
        m["xt"] = np.ascontiguousarray(xb.T, dtype=dt_in)
        m["xth"] = np.ascontiguousarray(xb[h * SH:(h + 1) * SH, :].T, dtype=dt_in)
        in_maps.append(m)
    return in_maps


def run(inputs: dict, cfg: Cfg = PROD, trace: bool = False):
    from concourse.bass_utils import run_bass_kernel_spmd

    nc = build_nc(cfg)
    in_maps = make_in_maps(cfg, inputs["x"], inputs["Wq"], inputs["bq"],
                           inputs["Wk"], inputs["bk"], inputs["Wv"], inputs["bv"])
    res = run_bass_kernel_spmd(nc, in_maps, list(range(cfg.n_cores)), trace=trace)
    out = assemble(cfg, [r["y"] for r in res.results])
    return out, res


def assemble(cfg: Cfg, ys):
    """Core 2b holds q-rows [0, S/2), core 2b+1 holds [S/2, S) of batch b."""
    B, S, D = cfg.B, cfg.S, cfg.D
    out = np.empty((B, S, D), dtype=np.float32)
    for b in range(B):
        out[b, : cfg.SH] = ys[2 * b]
        out[b, cfg.SH:] = ys[2 * b + 1]
    return out


def kernel(**inputs) -> np.ndarray:
    out, _ = run(inputs, PROD, trace=False)
    return out


# revision 24
# speedup vs baseline: 1.2107x; 1.0028x over previous
"""Multi-head-free attention (softmax over the QUERY axis) on 8 trn2 NeuronCores.

Problem: x:[4,2048,1024], Wq/Wk/Wv:[1024,1024], bq/bk/bv:[1024]
    q = x@Wq+bq ; k = x@Wk+bk ; v = x@Wv+bv
    scores = einsum('bqd,bkd->bqk', q, k) / 32
    attn   = softmax(scores, axis=1)          # over q (dim 1)!
    out    = einsum('bqk,bkv->bqv', attn, v)

Sharding: 4 batches x 2-way split of the QUERY axis across 8 cores
(core c -> batch c//2, query-half c%2).  Each core computes its q-rows
of the output against the FULL key range, so the only cross-core
dependency is the softmax denominator D[k] = sum_q exp(s[q,k]) -- an
8 KB pairwise AllReduce that hides behind the V-projection phase.
There is no output collective: each core DMAs its q-rows directly.

All matmuls run as float32r (full PE rate at N=512 moving dim, fp32
storage).  The attn*V contraction runs in bf16 (attn weights + V), with
fp32 PSUM accumulation.
"""

import sys

if "/opt/trn_rl_repo" not in sys.path:
    sys.path.insert(0, "/opt/trn_rl_repo")

import numpy as np

P = 128  # SBUF partitions


class Cfg:
    def __init__(self, B=4, S=2048, E=1024, D=1024, NB=512, n_cores=8, mm="f32r"):
        self.B, self.S, self.E, self.D, self.NB = B, S, E, D, NB
        self.mm = mm
        self.SH = S // 2          # per-core query-half length
        self.NE = E // P          # e (contraction) tiles
        self.ND = D // P          # d tiles
        self.NQb = self.SH // NB  # q 512-blocks (my half)
        self.NKb = S // NB        # k 512-blocks (full)
        self.NKt = S // P         # k 128-tiles (full)
        self.NQt = self.SH // P   # q 128-tiles (my half)
        self.NDVB = D // NB       # dv 512-blocks
        self.n_cores = n_cores
        self.groups = [[2 * i, 2 * i + 1] for i in range(n_cores // 2)]


PROD = Cfg()


def build_nc(cfg: Cfg):
    from concourse import bacc, bass, mybir, tile

    f32 = mybir.dt.float32
    f32r = mybir.dt.float32r
    bf16 = mybir.dt.bfloat16
    AF = mybir.ActivationFunctionType
    X = mybir.AxisListType.X
    ts = bass.ts

    B, S, E, D, NB = cfg.B, cfg.S, cfg.E, cfg.D, cfg.NB
    SH, NE, ND = cfg.SH, cfg.NE, cfg.ND
    NQb, NKb, NKt, NQt, NDVB = cfg.NQb, cfg.NKb, cfg.NKt, cfg.NQt, cfg.NDVB
    KT_PER_B = NB // P
    inv_sqrt_d = 1.0 / float(np.sqrt(np.float32(D)))

    nc = bacc.Bacc(None, num_devices=cfg.n_cores)
    dt_in = bf16 if cfg.mm == "bf16" else f32r

    # Per-core inputs (host pre-shards / pre-transposes).
    # xt: X^T for the whole batch (K and V read all tokens).
    # xth: X^T columns of MY query-half (Q reads only those).
    xt_d = nc.declare_dram_parameter("xt", [E, S], dt_in, isOutput=False)
    xth_d = nc.declare_dram_parameter("xth", [E, SH], dt_in, isOutput=False)
    wq_d = nc.declare_dram_parameter("wq", [E, D], dt_in, isOutput=False)
    wk_d = nc.declare_dram_parameter("wk", [E, D], dt_in, isOutput=False)
    wv_d = nc.declare_dram_parameter("wv", [E, D], dt_in, isOutput=False)
    bq_d = nc.declare_dram_parameter("bq", [D, 1], f32, isOutput=False)
    bk_d = nc.declare_dram_parameter("bk", [D, 1], f32, isOutput=False)
    bvb_d = nc.declare_dram_parameter("bvb", [P, D], f32, isOutput=False)
    y_d = nc.declare_dram_parameter("y", [SH, D], f32, isOutput=True)

    def r(ap):  # tensors feeding the PE are already dt_in (f32r or bf16)
        return ap

    with tile.TileContext(nc) as tc:
        with (
            tc.tile_pool(name="w", bufs=8 * ND + 1) as w_pool,
            tc.tile_pool(name="x", bufs=11) as x_pool,
            tc.tile_pool(name="qt", bufs=1) as qt_pool,
            tc.tile_pool(name="ktw", bufs=10) as ktw_pool,
            tc.tile_pool(name="pt", bufs=1) as pt_pool,
            tc.tile_pool(name="v", bufs=1) as v_pool,
            tc.tile_pool(name="wv", bufs=NE) as wv_pool,
            tc.tile_pool(name="ob", bufs=3) as out_pool,
            tc.tile_pool(name="small", bufs=1) as small_pool,
            tc.tile_pool(name="ps", bufs=8, space="PSUM") as ps_pool,
            tc.tile_pool(name="dram", bufs=1, space="DRAM") as dram_pool,
        ):
            ar_in = dram_pool.tile([P, NKt], f32)
            ar_out = dram_pool.tile([P, NKt], f32)

            # ---- constants ----
            bvb_t = small_pool.tile([P, D], f32, tag="bvb")
            nc.scalar.dma_start(bvb_t[:], bvb_d[:])
            bq_t, bk_t = [], []
            for dt in range(ND):
                bqt = small_pool.tile([P, 1], f32, tag=f"bq{dt}")
                nc.scalar.dma_start(bqt[:], bq_d[ts(dt, P), :])
                bq_t.append(bqt)
                bkt = small_pool.tile([P, 1], f32, tag=f"bk{dt}")
                nc.scalar.dma_start(bkt[:], bk_d[ts(dt, P), :])
                bk_t.append(bkt)

            # ---- phase Q: QT[dt][d, q] = Q[q, d]^T for MY q-half (resident) ----
            qt_tiles = []
            for dt in range(ND):
                q = qt_pool.tile([P, SH], dt_in, tag=f"qt{dt}", name=f"qtt{dt}")
                qt_tiles.append(q)
            wq_t = {}
            for qb in range(NQb):
                xq = []
                for et in range(NE):
                    t = x_pool.tile([P, NB], dt_in, tag="x", name=f"xq{qb}_{et}")
                    nc.sync.dma_start(t[:], xth_d[ts(et, P), ts(qb, NB)])
                    xq.append(t)
                for dt in range(ND):
                    if qb == 0:
                        for et in range(NE):
                            w = w_pool.tile([P, P], dt_in, tag="w", name=f"wq{et}_{dt}")
                            nc.scalar.dma_start(w[:], wq_d[ts(et, P), ts(dt, P)])
                            wq_t[(et, dt)] = w
                    ps = ps_pool.tile([P, NB], f32, tag="ps", name="psq")
                    for et in range(NE):
                        nc.tensor.matmul(
                            ps[:], r(wq_t[(et, dt)][:]), r(xq[et][:]),
                            start=(et == 0), stop=(et == NE - 1),
                        )
                    nc.scalar.activation(
                        qt_tiles[dt][:, ts(qb, NB)], ps[:], AF.Identity, bias=bq_t[dt][:]
                    )

            # prefetch Wv's first dv-half early (bandwidth is idle during K+S)
            wv_t = {}
            for et in range(NE):
                w = wv_pool.tile([P, NB], dt_in, tag="wv", name=f"wv{et}_0")
                nc.scalar.dma_start(w[:], wv_d[ts(et, P), ts(0, NB)])
                wv_t[(et, 0)] = w

            # ---- phase K+S fused: per 512-token k-block, project K then score ----
            # PT[kt][k, q] = exp(scores[q, k] / sqrt(D)) for all k, my q-half
            pt_tiles = []
            rs_t = []
            for kt in range(NKt):
                ptt = pt_pool.tile([P, SH], bf16, tag=f"pt{kt}", name=f"ptt{kt}")
                pt_tiles.append(ptt)
                rst = small_pool.tile([P, NQb], f32, tag=f"rs{kt}", name=f"rst{kt}")
                rs_t.append(rst)
            rs_all = small_pool.tile([P, NKt], f32, tag="rsall")
            wk_t = {}
            for kb in range(NKb):
                xk = []
                for et in range(NE):
                    t = x_pool.tile([P, NB], dt_in, tag="x", name=f"xk{kb}_{et}")
                    nc.sync.dma_start(t[:], xt_d[ts(et, P), ts(kb, NB)])
                    xk.append(t)
                ktw = []
                for dt in range(ND):
                    if kb == 0:
                        for et in range(NE):
                            w = w_pool.tile([P, P], dt_in, tag="w", name=f"wk{et}_{dt}")
                            nc.scalar.dma_start(w[:], wk_d[ts(et, P), ts(dt, P)])
                            wk_t[(et, dt)] = w
                    ps = ps_pool.tile([P, NB], f32, tag="ps", name="psk")
                    for et in range(NE):
                        nc.tensor.matmul(
                            ps[:], r(wk_t[(et, dt)][:]), r(xk[et][:]),
                            start=(et == 0), stop=(et == NE - 1),
                        )
                    kw = ktw_pool.tile([P, NB], dt_in, tag="ktw", name=f"ktw{kb}_{dt}")
                    nc.scalar.activation(kw[:], ps[:], AF.Identity, bias=bk_t[dt][:])
                    ktw.append(kw)
                for kt4 in range(KT_PER_B):
                    kt = kb * KT_PER_B + kt4
                    for qb in range(NQb):
                        ps = ps_pool.tile([P, NB], f32, tag="ps", name="pss")
                        for dt in range(ND):
                            nc.tensor.matmul(
                                ps[:], r(ktw[dt][:, ts(kt4, P)]),
                                r(qt_tiles[dt][:, ts(qb, NB)]),
                                start=(dt == 0), stop=(dt == ND - 1),
                            )
                        nc.scalar.activation(
                            pt_tiles[kt][:, ts(qb, NB)], ps[:], AF.Exp,
                            scale=inv_sqrt_d,
                            accum_out=rs_t[kt][:, qb:qb + 1],
                        )
                    nc.vector.reduce_sum(rs_all[:, kt:kt + 1], rs_t[kt][:], axis=X)

            # ---- softmax denominators: pairwise 8KB AllReduce, then 1/D ----
            nc.sync.dma_start(ar_in[:], rs_all[:])
            nc.gpsimd.collective_compute(
                "AllReduce",
                mybir.AluOpType.add,
                replica_groups=cfg.groups,
                ins=[ar_in[:].opt()],
                outs=[ar_out[:].opt()],
            )
            rsum_t = small_pool.tile([P, NKt], f32, tag="rsum")
            nc.sync.dma_start(rsum_t[:], ar_out[:])
            rcp_all = small_pool.tile([P, NKt], f32, tag="rcp")
            nc.vector.reciprocal(rcp_all[:], rsum_t[:])

            # ---- phase V: V[kt][k, dv] = X@Wv + bv (full tokens; no rcp dep) ----
            v_tiles = []
            for kt in range(NKt):
                vt = v_pool.tile([P, D], bf16, tag=f"v{kt}", name=f"vt{kt}")
                v_tiles.append(vt)
            # Wv's second dv-half (if any) reuses QT slots (QT dead after K+S)
            for dvb in range(1, NDVB):
                for et in range(NE):
                    w = qt_pool.tile([P, NB], dt_in, tag=f"qt{et}", name=f"wv{et}_{dvb}")
                    nc.scalar.dma_start(w[:], wv_d[ts(et, P), ts(dvb, NB)])
                    wv_t[(et, dvb)] = w
            for kb in range(NKb):
                xv = []
                for et in range(NE):
                    t = x_pool.tile([P, NB], dt_in, tag="x", name=f"xv{kb}_{et}")
                    nc.sync.dma_start(t[:], xt_d[ts(et, P), ts(kb, NB)])
                    xv.append(t)
                for kt4 in range(KT_PER_B):
                    kt = kb * KT_PER_B + kt4
                    for dvb in range(NDVB):
                        ps = ps_pool.tile([P, NB], f32, tag="ps", name="psv")
                        for et in range(NE):
                            nc.tensor.matmul(
                                ps[:], r(xv[et][:, ts(kt4, P)]),
                                r(wv_t[(et, dvb)][:]),
                                start=(et == 0), stop=(et == NE - 1),
                            )
                        nc.vector.tensor_add(
                            v_tiles[kt][:, ts(dvb, NB)], ps[:], bvb_t[:, ts(dvb, NB)]
                        )

            # attn = PT * (1/D[k]) -- per-partition (k) scale, in place, on DVE
            for kt in range(NKt):
                nc.vector.tensor_scalar_mul(
                    pt_tiles[kt][:], pt_tiles[kt][:], rcp_all[:, kt:kt + 1]
                )

            # ---- phase AV: y[q, dv] = sum_k attn[k,q] * V[k,dv]; direct DMA out ----
            for qt in range(NQt):
                for dvb in range(NDVB):
                    ps = ps_pool.tile([P, NB], f32, tag="ps", name="psav")
                    for kt in range(NKt):
                        nc.tensor.matmul(
                            ps[:], pt_tiles[kt][:, ts(qt, P)],
                            v_tiles[kt][:, ts(dvb, NB)],
                            start=(kt == 0), stop=(kt == NKt - 1),
                        )
                    ob = out_pool.tile([P, NB], f32, tag="ob", name="ob")
                    nc.scalar.copy(ob[:], ps[:])
                    nc.sync.dma_start(y_d[ts(qt, P), ts(dvb, NB)], ob[:])

    nc.compile()
    return nc


def make_in_maps(cfg: Cfg, x, Wq, bq, Wk, bk, Wv, bv):
    SH = cfg.SH
    f32 = np.float32
    if cfg.mm == "bf16":
        import ml_dtypes
        dt_in = ml_dtypes.bfloat16
    else:
        dt_in = f32
    in_maps = []
    shared = {
        "wq": np.ascontiguousarray(Wq, dtype=dt_in),
        "wk": np.ascontiguousarray(Wk, dtype=dt_in),
        "wv": np.ascontiguousarray(Wv, dtype=dt_in),
        "bq": np.ascontiguousarray(np.reshape(bq, (-1, 1)), dtype=f32),
        "bk": np.ascontiguousarray(np.reshape(bk, (-1, 1)), dtype=f32),
        "bvb": np.ascontiguousarray(np.broadcast_to(np.reshape(bv, (1, -1)), (128, len(np.ravel(bv)))), dtype=f32),
    }
    for c in range(cfg.n_cores):
        b, h = c // 2, c % 2
        xb = np.asarray(x[b], dtype=f32)
        m = dict(shared)
        m["xt"] = np.ascontiguousarray(xb.T, dtype=dt_in)
        m["xth"] = np.ascontiguousarray(xb[h * SH:(h + 1) * SH, :].T, dtype=dt_in)
        in_maps.append(m)
    return in_maps


def run(inputs: dict, cfg: Cfg = PROD, trace: bool = False):
    from concourse.bass_utils import run_bass_kernel_spmd

    nc = build_nc(cfg)
    in_maps = make_in_maps(cfg, inputs["x"], inputs["Wq"], inputs["bq"],
                           inputs["Wk"], inputs["bk"], inputs["Wv"], inputs["bv"])
    res = run_bass_kernel_spmd(nc, in_maps, list(range(cfg.n_cores)), trace=trace)
    out = assemble(cfg, [r["y"] for r in res.results])
    return out, res


def assemble(cfg: Cfg, ys):
    """Core 2b holds q-rows [0, S/2), core 2b+1 holds [S/2, S) of batch b."""
    B, S, D = cfg.B, cfg.S, cfg.D
    out = np.empty((B, S, D), dtype=np.float32)
    for b in range(B):
        out[b, : cfg.SH] = ys[2 * b]
        out[b, cfg.SH:] = ys[2 * b + 1]
    return out


def kernel(**inputs) -> np.ndarray:
    out, _ = run(inputs, PROD, trace=False)
    return out


# revision 25
# speedup vs baseline: 1.2198x; 1.0075x over previous
"""Multi-head-free attention (softmax over the QUERY axis) on 8 trn2 NeuronCores.

Problem: x:[4,2048,1024], Wq/Wk/Wv:[1024,1024], bq/bk/bv:[1024]
    q = x@Wq+bq ; k = x@Wk+bk ; v = x@Wv+bv
    scores = einsum('bqd,bkd->bqk', q, k) / 32
    attn   = softmax(scores, axis=1)          # over q (dim 1)!
    out    = einsum('bqk,bkv->bqv', attn, v)

Sharding: 4 batches x 2-way split of the QUERY axis across 8 cores
(core c -> batch c//2, query-half c%2).  Each core computes its q-rows
of the output against the FULL key range, so the only cross-core
dependency is the softmax denominator D[k] = sum_q exp(s[q,k]) -- an
8 KB pairwise AllReduce that hides behind the V-projection phase.
There is no output collective: each core DMAs its q-rows directly.

All matmuls run as float32r (full PE rate at N=512 moving dim, fp32
storage).  The attn*V contraction runs in bf16 (attn weights + V), with
fp32 PSUM accumulation.
"""

import sys

if "/opt/trn_rl_repo" not in sys.path:
    sys.path.insert(0, "/opt/trn_rl_repo")

import numpy as np

P = 128  # SBUF partitions


class Cfg:
    def __init__(self, B=4, S=2048, E=1024, D=1024, NB=512, n_cores=8, mm="f32r"):
        self.B, self.S, self.E, self.D, self.NB = B, S, E, D, NB
        self.mm = mm
        self.SH = S // 2          # per-core query-half length
        self.NE = E // P          # e (contraction) tiles
        self.ND = D // P          # d tiles
        self.NQb = self.SH // NB  # q 512-blocks (my half)
        self.NKb = S // NB        # k 512-blocks (full)
        self.NKt = S // P         # k 128-tiles (full)
        self.NQt = self.SH // P   # q 128-tiles (my half)
        self.NDVB = D // NB       # dv 512-blocks
        self.n_cores = n_cores
        self.groups = [[2 * i, 2 * i + 1] for i in range(n_cores // 2)]


PROD = Cfg()


def build_nc(cfg: Cfg):
    from concourse import bacc, bass, mybir, tile

    f32 = mybir.dt.float32
    f32r = mybir.dt.float32r
    bf16 = mybir.dt.bfloat16
    AF = mybir.ActivationFunctionType
    X = mybir.AxisListType.X
    ts = bass.ts

    B, S, E, D, NB = cfg.B, cfg.S, cfg.E, cfg.D, cfg.NB
    SH, NE, ND = cfg.SH, cfg.NE, cfg.ND
    NQb, NKb, NKt, NQt, NDVB = cfg.NQb, cfg.NKb, cfg.NKt, cfg.NQt, cfg.NDVB
    KT_PER_B = NB // P
    inv_sqrt_d = 1.0 / float(np.sqrt(np.float32(D)))

    nc = bacc.Bacc(None, num_devices=cfg.n_cores)
    dt_in = bf16 if cfg.mm == "bf16" else f32r

    # Per-core inputs (host pre-shards / pre-transposes).
    # xt: X^T for the whole batch (K and V read all tokens).
    # xth: X^T columns of MY query-half (Q reads only those).
    xt_d = nc.declare_dram_parameter("xt", [E, S], dt_in, isOutput=False)
    xth_d = nc.declare_dram_parameter("xth", [E, SH], dt_in, isOutput=False)
    wq_d = nc.declare_dram_parameter("wq", [E, D], dt_in, isOutput=False)
    wk_d = nc.declare_dram_parameter("wk", [E, D], dt_in, isOutput=False)
    wv_d = nc.declare_dram_parameter("wv", [E, D], dt_in, isOutput=False)
    bq_d = nc.declare_dram_parameter("bq", [D, 1], f32, isOutput=False)
    bk_d = nc.declare_dram_parameter("bk", [D, 1], f32, isOutput=False)
    bvb_d = nc.declare_dram_parameter("bvb", [P, D], f32, isOutput=False)
    y_d = nc.declare_dram_parameter("y", [SH, D], f32, isOutput=True)

    def r(ap):  # tensors feeding the PE are already dt_in (f32r or bf16)
        return ap

    with tile.TileContext(nc) as tc:
        with (
            tc.tile_pool(name="w", bufs=8 * ND + 1) as w_pool,
            tc.tile_pool(name="x", bufs=14) as x_pool,
            tc.tile_pool(name="qt", bufs=1) as qt_pool,
            tc.tile_pool(name="ktw", bufs=9) as ktw_pool,
            tc.tile_pool(name="pt", bufs=1) as pt_pool,
            tc.tile_pool(name="v", bufs=1) as v_pool,
            tc.tile_pool(name="wv", bufs=NE) as wv_pool,
            tc.tile_pool(name="ob", bufs=3) as out_pool,
            tc.tile_pool(name="small", bufs=1) as small_pool,
            tc.tile_pool(name="ps", bufs=8, space="PSUM") as ps_pool,
            tc.tile_pool(name="dram", bufs=1, space="DRAM") as dram_pool,
        ):
            ar_in = dram_pool.tile([P, NKt], f32)
            ar_out = dram_pool.tile([P, NKt], f32)

            # ---- constants (only bq is needed immediately) ----
            bq_t = []
            for dt in range(ND):
                bqt = small_pool.tile([P, 1], f32, tag=f"bq{dt}")
                nc.scalar.dma_start(bqt[:], bq_d[ts(dt, P), :])
                bq_t.append(bqt)

            # ---- phase Q: QT[dt][d, q] = Q[q, d]^T for MY q-half (resident) ----
            qt_tiles = []
            for dt in range(ND):
                q = qt_pool.tile([P, SH], dt_in, tag=f"qt{dt}", name=f"qtt{dt}")
                qt_tiles.append(q)
            wq_t = {}
            for qb in range(NQb):
                xq = []
                for et in range(NE):
                    t = x_pool.tile([P, NB], dt_in, tag="x", name=f"xq{qb}_{et}")
                    nc.sync.dma_start(t[:], xth_d[ts(et, P), ts(qb, NB)])
                    xq.append(t)
                for dt in range(ND):
                    if qb == 0:
                        for et in range(NE):
                            w = w_pool.tile([P, P], dt_in, tag="w", name=f"wq{et}_{dt}")
                            nc.scalar.dma_start(w[:], wq_d[ts(et, P), ts(dt, P)])
                            wq_t[(et, dt)] = w
                    ps = ps_pool.tile([P, NB], f32, tag="ps", name="psq")
                    for et in range(NE):
                        nc.tensor.matmul(
                            ps[:], r(wq_t[(et, dt)][:]), r(xq[et][:]),
                            start=(et == 0), stop=(et == NE - 1),
                        )
                    nc.scalar.activation(
                        qt_tiles[dt][:, ts(qb, NB)], ps[:], AF.Identity, bias=bq_t[dt][:]
                    )

            # prefetch Wv's first dv-half early (bandwidth is idle during K+S)
            wv_t = {}
            for et in range(NE):
                w = wv_pool.tile([P, NB], dt_in, tag="wv", name=f"wv{et}_0")
                nc.scalar.dma_start(w[:], wv_d[ts(et, P), ts(0, NB)])
                wv_t[(et, 0)] = w

            bk_t = []
            for dt in range(ND):
                bkt = small_pool.tile([P, 1], f32, tag=f"bk{dt}")
                nc.scalar.dma_start(bkt[:], bk_d[ts(dt, P), :])
                bk_t.append(bkt)

            # ---- phase K+S fused: per 512-token k-block, project K then score ----
            # PT[kt][k, q] = exp(scores[q, k] / sqrt(D)) for all k, my q-half
            pt_tiles = []
            rs_t = []
            for kt in range(NKt):
                ptt = pt_pool.tile([P, SH], bf16, tag=f"pt{kt}", name=f"ptt{kt}")
                pt_tiles.append(ptt)
                rst = small_pool.tile([P, NQb], f32, tag=f"rs{kt}", name=f"rst{kt}")
                rs_t.append(rst)
            rs_all = small_pool.tile([P, NKt], f32, tag="rsall")
            wk_t = {}
            for kb in range(NKb):
                xk = []
                for et in range(NE):
                    t = x_pool.tile([P, NB], dt_in, tag="x", name=f"xk{kb}_{et}")
                    nc.sync.dma_start(t[:], xt_d[ts(et, P), ts(kb, NB)])
                    xk.append(t)
                ktw = []
                for dt in range(ND):
                    if kb == 0:
                        for et in range(NE):
                            w = w_pool.tile([P, P], dt_in, tag="w", name=f"wk{et}_{dt}")
                            nc.scalar.dma_start(w[:], wk_d[ts(et, P), ts(dt, P)])
                            wk_t[(et, dt)] = w
                    ps = ps_pool.tile([P, NB], f32, tag="ps", name="psk")
                    for et in range(NE):
                        nc.tensor.matmul(
                            ps[:], r(wk_t[(et, dt)][:]), r(xk[et][:]),
                            start=(et == 0), stop=(et == NE - 1),
                        )
                    kw = ktw_pool.tile([P, NB], dt_in, tag="ktw", name=f"ktw{kb}_{dt}")
                    nc.scalar.activation(kw[:], ps[:], AF.Identity, bias=bk_t[dt][:])
                    ktw.append(kw)
                for kt4 in range(KT_PER_B):
                    kt = kb * KT_PER_B + kt4
                    for qb in range(NQb):
                        ps = ps_pool.tile([P, NB], f32, tag="ps", name="pss")
                        for dt in range(ND):
                            nc.tensor.matmul(
                                ps[:], r(ktw[dt][:, ts(kt4, P)]),
                                r(qt_tiles[dt][:, ts(qb, NB)]),
                                start=(dt == 0), stop=(dt == ND - 1),
                            )
                        nc.scalar.activation(
                            pt_tiles[kt][:, ts(qb, NB)], ps[:], AF.Exp,
                            scale=inv_sqrt_d,
                            accum_out=rs_t[kt][:, qb:qb + 1],
                        )
                    nc.vector.reduce_sum(rs_all[:, kt:kt + 1], rs_t[kt][:], axis=X)

            # ---- softmax denominators: pairwise 8KB AllReduce, then 1/D ----
            nc.sync.dma_start(ar_in[:], rs_all[:])
            nc.gpsimd.collective_compute(
                "AllReduce",
                mybir.AluOpType.add,
                replica_groups=cfg.groups,
                ins=[ar_in[:].opt()],
                outs=[ar_out[:].opt()],
            )
            rsum_t = small_pool.tile([P, NKt], f32, tag="rsum")
            nc.sync.dma_start(rsum_t[:], ar_out[:])
            rcp_all = small_pool.tile([P, NKt], f32, tag="rcp")
            nc.vector.reciprocal(rcp_all[:], rsum_t[:])

            bvb_t = small_pool.tile([P, D], f32, tag="bvb")
            nc.scalar.dma_start(bvb_t[:], bvb_d[:])

            # ---- phase V: V[kt][k, dv] = X@Wv + bv (full tokens; no rcp dep) ----
            v_tiles = []
            for kt in range(NKt):
                vt = v_pool.tile([P, D], bf16, tag=f"v{kt}", name=f"vt{kt}")
                v_tiles.append(vt)
            # Wv's second dv-half (if any) reuses QT slots (QT dead after K+S)
            for dvb in range(1, NDVB):
                for et in range(NE):
                    w = qt_pool.tile([P, NB], dt_in, tag=f"qt{et}", name=f"wv{et}_{dvb}")
                    nc.scalar.dma_start(w[:], wv_d[ts(et, P), ts(dvb, NB)])
                    wv_t[(et, dvb)] = w
            for kb in range(NKb):
                xv = []
                for et in range(NE):
                    t = x_pool.tile([P, NB], dt_in, tag="x", name=f"xv{kb}_{et}")
                    nc.sync.dma_start(t[:], xt_d[ts(et, P), ts(kb, NB)])
                    xv.append(t)
                for kt4 in range(KT_PER_B):
                    kt = kb * KT_PER_B + kt4
                    for dvb in range(NDVB):
                        ps = ps_pool.tile([P, NB], f32, tag="ps", name="psv")
                        for et in range(NE):
                            nc.tensor.matmul(
                                ps[:], r(xv[et][:, ts(kt4, P)]),
                                r(wv_t[(et, dvb)][:]),
                                start=(et == 0), stop=(et == NE - 1),
                            )
                        nc.vector.tensor_add(
                            v_tiles[kt][:, ts(dvb, NB)], ps[:], bvb_t[:, ts(dvb, NB)]
                        )

            # attn = PT * (1/D[k]) -- per-partition (k) scale, in place, on DVE
            for kt in range(NKt):
                nc.vector.tensor_scalar_mul(
                    pt_tiles[kt][:], pt_tiles[kt][:], rcp_all[:, kt:kt + 1]
                )

            # ---- phase AV: y[q, dv] = sum_k attn[k,q] * V[k,dv]; direct DMA out ----
            for qt in range(NQt):
                for dvb in range(NDVB):
                    ps = ps_pool.tile([P, NB], f32, tag="ps", name="psav")
                    for kt in range(NKt):
                        nc.tensor.matmul(
                            ps[:], pt_tiles[kt][:, ts(qt, P)],
                            v_tiles[kt][:, ts(dvb, NB)],
                            start=(kt == 0), stop=(kt == NKt - 1),
                        )
                    ob = out_pool.tile([P, NB], f32, tag="ob", name="ob")
                    nc.scalar.copy(ob[:], ps[:])
                    nc.sync.dma_start(y_d[ts(qt, P), ts(dvb, NB)], ob[:])

    nc.compile()
    return nc


def make_in_maps(cfg: Cfg, x, Wq, bq, Wk, bk, Wv, bv):
    SH = cfg.SH
    f32 = np.float32
    if cfg.mm == "bf16":
        import ml_dtypes
        dt_in = ml_dtypes.bfloat16
    else:
        dt_in = f32
    in_maps = []
    shared = {
        "wq": np.ascontiguousarray(Wq, dtype=dt_in),
        "wk": np.ascontiguousarray(Wk, dtype=dt_in),
        "wv": np.ascontiguousarray(Wv, dtype=dt_in),
        "bq": np.ascontiguousarray(np.reshape(bq, (-1, 1)), dtype=f32),
        "bk": np.ascontiguousarray(np.reshape(bk, (-1, 1)), dtype=f32),
        "bvb": np.ascontiguousarray(np.broadcast_to(np.reshape(bv, (1, -1)), (128, len(np.ravel(bv)))), dtype=f32),
    }
    for c in range(cfg.n_cores):
        b, h = c // 2, c % 2
        xb = np.asarray(x[b], dtype=f32)
        m = dict(shared)
        m["xt"] = np.ascontiguousarray(xb.T, dtype=dt_in)
        m["xth"] = np.ascontiguousarray(xb[h * SH:(h + 1) * SH, :].T, dtype=dt_in)
        in_maps.append(m)
    return in_maps


def run(inputs: dict, cfg: Cfg = PROD, trace: bool = False):
    from concourse.bass_utils import run_bass_kernel_spmd

    nc = build_nc(cfg)
    in_maps = make_in_maps(cfg, inputs["x"], inputs["Wq"], inputs["bq"],
                           inputs["Wk"], inputs["bk"], inputs["Wv"], inputs["bv"])
    res = run_bass_kernel_spmd(nc, in_maps, list(range(cfg.n_cores)), trace=trace)
    out = assemble(cfg, [r["y"] for r in res.results])
    return out, res


def assemble(cfg: Cfg, ys):
    """Core 2b holds q-rows [0, S/2), core 2b+1 holds [S/2, S) of batch b."""
    B, S, D = cfg.B, cfg.S, cfg.D
    out = np.empty((B, S, D), dtype=np.float32)
    for b in range(B):
        out[b, : cfg.SH] = ys[2 * b]
        out[b, cfg.SH:] = ys[2 * b + 1]
    return out


def kernel(**inputs) -> np.ndarray:
    out, _ = run(inputs, PROD, trace=False)
    return out


# revision 26
# speedup vs baseline: 1.3397x; 1.0983x over previous
"""Multi-head-free attention (softmax over the QUERY axis) on 8 trn2 NeuronCores.

Problem: x:[4,2048,1024], Wq/Wk/Wv:[1024,1024], bq/bk/bv:[1024]
    q = x@Wq+bq ; k = x@Wk+bk ; v = x@Wv+bv
    scores = einsum('bqd,bkd->bqk', q, k) / 32
    attn   = softmax(scores, axis=1)          # over q (dim 1)!
    out    = einsum('bqk,bkv->bqv', attn, v)

Sharding: 4 batches x 2-way split of the QUERY axis across 8 cores
(core c -> batch c//2, query-half c%2).  Each core computes its q-rows
of the output against the FULL key range, so the only cross-core
dependency is the softmax denominator D[k] = sum_q exp(s[q,k]) -- an
8 KB pairwise AllReduce that hides behind the V-projection phase.
There is no output collective: each core DMAs its q-rows directly.

All matmuls run as float32r (full PE rate at N=512 moving dim, fp32
storage).  The attn*V contraction runs in bf16 (attn weights + V), with
fp32 PSUM accumulation.
"""

import sys

if "/opt/trn_rl_repo" not in sys.path:
    sys.path.insert(0, "/opt/trn_rl_repo")

import numpy as np

P = 128  # SBUF partitions


class Cfg:
    def __init__(self, B=4, S=2048, E=1024, D=1024, NB=512, n_cores=8, mm="f32r"):
        self.B, self.S, self.E, self.D, self.NB = B, S, E, D, NB
        self.mm = mm
        self.SH = S // 2          # per-core query-half length
        self.NE = E // P          # e (contraction) tiles
        self.ND = D // P          # d tiles
        self.NQb = self.SH // NB  # q 512-blocks (my half)
        self.NKb = S // NB        # k 512-blocks (full)
        self.NKt = S // P         # k 128-tiles (full)
        self.NQt = self.SH // P   # q 128-tiles (my half)
        self.NDVB = D // NB       # dv 512-blocks
        self.n_cores = n_cores
        self.groups = [[2 * i, 2 * i + 1] for i in range(n_cores // 2)]


PROD = Cfg()


def build_nc(cfg: Cfg):
    from concourse import bacc, bass, mybir, tile

    f32 = mybir.dt.float32
    f32r = mybir.dt.float32r
    bf16 = mybir.dt.bfloat16
    AF = mybir.ActivationFunctionType
    X = mybir.AxisListType.X
    ts = bass.ts

    B, S, E, D, NB = cfg.B, cfg.S, cfg.E, cfg.D, cfg.NB
    SH, NE, ND = cfg.SH, cfg.NE, cfg.ND
    NQb, NKb, NKt, NQt, NDVB = cfg.NQb, cfg.NKb, cfg.NKt, cfg.NQt, cfg.NDVB
    KT_PER_B = NB // P
    inv_sqrt_d = 1.0 / float(np.sqrt(np.float32(D)))

    nc = bacc.Bacc(None, num_devices=cfg.n_cores)
    dt_in = bf16 if cfg.mm == "bf16" else f32r

    # Per-core inputs (host pre-shards / pre-transposes).
    # xt: X^T for the whole batch (K and V read all tokens).
    # xth: X^T columns of MY query-half (Q reads only those).
    xt_d = nc.declare_dram_parameter("xt", [E, S], dt_in, isOutput=False)
    xth_d = nc.declare_dram_parameter("xth", [E, SH], dt_in, isOutput=False)
    wq_d = nc.declare_dram_parameter("wq", [E, D], dt_in, isOutput=False)
    wk_d = nc.declare_dram_parameter("wk", [E, D], dt_in, isOutput=False)
    wv_d = nc.declare_dram_parameter("wv", [E, D], dt_in, isOutput=False)
    bq_d = nc.declare_dram_parameter("bq", [D, 1], f32, isOutput=False)
    bk_d = nc.declare_dram_parameter("bk", [D, 1], f32, isOutput=False)
    bvb_d = nc.declare_dram_parameter("bvb", [P, D], f32, isOutput=False)
    y_d = nc.declare_dram_parameter("y", [SH, D], f32, isOutput=True)

    def r(ap):  # tensors feeding the PE are already dt_in (f32r or bf16)
        return ap

    with tile.TileContext(nc) as tc:
        with (
            tc.tile_pool(name="w", bufs=NE + 1) as w_pool,
            tc.tile_pool(name="x", bufs=13) as x_pool,
            tc.tile_pool(name="qt", bufs=1) as qt_pool,
            tc.tile_pool(name="ktw", bufs=8) as ktw_pool,
            tc.tile_pool(name="pt", bufs=1) as pt_pool,
            tc.tile_pool(name="v", bufs=1) as v_pool,
            tc.tile_pool(name="wv", bufs=NE) as wv_pool,
            tc.tile_pool(name="ob", bufs=3) as out_pool,
            tc.tile_pool(name="small", bufs=1) as small_pool,
            tc.tile_pool(name="ps", bufs=8, space="PSUM") as ps_pool,
            tc.tile_pool(name="dram", bufs=1, space="DRAM") as dram_pool,
        ):
            ar_in = dram_pool.tile([P, NKt], f32)
            ar_out = dram_pool.tile([P, NKt], f32)

            # ---- constants (only bq is needed immediately) ----
            bq_t = []
            for dt in range(ND):
                bqt = small_pool.tile([P, 1], f32, tag=f"bq{dt}")
                nc.gpsimd.dma_start(bqt[:], bq_d[ts(dt, P), :])
                bq_t.append(bqt)

            # ---- phase Q: QT[dt][d, q] = Q[q, d]^T for MY q-half (resident) ----
            qt_tiles = []
            for dt in range(ND):
                q = qt_pool.tile([P, SH], dt_in, tag=f"qt{dt}", name=f"qtt{dt}")
                qt_tiles.append(q)
            wq_t = {}
            for qb in range(NQb):
                xq = []
                for et in range(NE):
                    t = x_pool.tile([P, NB], dt_in, tag="x", name=f"xq{qb}_{et}")
                    nc.sync.dma_start(t[:], xth_d[ts(et, P), ts(qb, NB)])
                    xq.append(t)
                if qb == 0:
                    for et in range(NE):
                        w = w_pool.tile([P, D], dt_in, tag="w", name=f"wq{et}")
                        for hf in range(D // NB):
                            nc.gpsimd.dma_start(w[:, ts(hf, NB)], wq_d[ts(et, P), ts(hf, NB)])
                        wq_t[et] = w
                for dt in range(ND):
                    ps = ps_pool.tile([P, NB], f32, tag="ps", name="psq")
                    for et in range(NE):
                        nc.tensor.matmul(
                            ps[:], r(wq_t[et][:, ts(dt, P)]), r(xq[et][:]),
                            start=(et == 0), stop=(et == NE - 1),
                        )
                    nc.scalar.activation(
                        qt_tiles[dt][:, ts(qb, NB)], ps[:], AF.Identity, bias=bq_t[dt][:]
                    )

            # prefetch Wv's first dv-half early (bandwidth is idle during K+S)
            wv_t = {}
            for et in range(NE):
                w = wv_pool.tile([P, NB], dt_in, tag="wv", name=f"wv{et}_0")
                nc.gpsimd.dma_start(w[:], wv_d[ts(et, P), ts(0, NB)])
                wv_t[(et, 0)] = w

            bk_t = []
            for dt in range(ND):
                bkt = small_pool.tile([P, 1], f32, tag=f"bk{dt}")
                nc.gpsimd.dma_start(bkt[:], bk_d[ts(dt, P), :])
                bk_t.append(bkt)

            # ---- phase K+S fused: per 512-token k-block, project K then score ----
            # PT[kt][k, q] = exp(scores[q, k] / sqrt(D)) for all k, my q-half
            pt_tiles = []
            rs_t = []
            for kt in range(NKt):
                ptt = pt_pool.tile([P, SH], bf16, tag=f"pt{kt}", name=f"ptt{kt}")
                pt_tiles.append(ptt)
                rst = small_pool.tile([P, NQb], f32, tag=f"rs{kt}", name=f"rst{kt}")
                rs_t.append(rst)
            rs_all = small_pool.tile([P, NKt], f32, tag="rsall")
            wk_t = {}
            for kb in range(NKb):
                xk = []
                for et in range(NE):
                    t = x_pool.tile([P, NB], dt_in, tag="x", name=f"xk{kb}_{et}")
                    nc.sync.dma_start(t[:], xt_d[ts(et, P), ts(kb, NB)])
                    xk.append(t)
                if kb == 0:
                    for et in range(NE):
                        w = w_pool.tile([P, D], dt_in, tag="w", name=f"wk{et}")
                        for hf in range(D // NB):
                            nc.gpsimd.dma_start(w[:, ts(hf, NB)], wk_d[ts(et, P), ts(hf, NB)])
                        wk_t[et] = w
                ktw = []
                for dt in range(ND):
                    ps = ps_pool.tile([P, NB], f32, tag="ps", name="psk")
                    for et in range(NE):
                        nc.tensor.matmul(
                            ps[:], r(wk_t[et][:, ts(dt, P)]), r(xk[et][:]),
                            start=(et == 0), stop=(et == NE - 1),
                        )
                    kw = ktw_pool.tile([P, NB], dt_in, tag="ktw", name=f"ktw{kb}_{dt}")
                    nc.scalar.activation(kw[:], ps[:], AF.Identity, bias=bk_t[dt][:])
                    ktw.append(kw)
                for kt4 in range(KT_PER_B):
                    kt = kb * KT_PER_B + kt4
                    for qb in range(NQb):
                        ps = ps_pool.tile([P, NB], f32, tag="ps", name="pss")
                        for dt in range(ND):
                            nc.tensor.matmul(
                                ps[:], r(ktw[dt][:, ts(kt4, P)]),
                                r(qt_tiles[dt][:, ts(qb, NB)]),
                                start=(dt == 0), stop=(dt == ND - 1),
                            )
                        nc.scalar.activation(
                            pt_tiles[kt][:, ts(qb, NB)], ps[:], AF.Exp,
                            scale=inv_sqrt_d,
                            accum_out=rs_t[kt][:, qb:qb + 1],
                        )
                    nc.vector.reduce_sum(rs_all[:, kt:kt + 1], rs_t[kt][:], axis=X)

            # ---- softmax denominators: pairwise 8KB AllReduce, then 1/D ----
            nc.sync.dma_start(ar_in[:], rs_all[:])
            nc.gpsimd.collective_compute(
                "AllReduce",
                mybir.AluOpType.add,
                replica_groups=cfg.groups,
                ins=[ar_in[:].opt()],
                outs=[ar_out[:].opt()],
            )
            rsum_t = small_pool.tile([P, NKt], f32, tag="rsum")
            nc.sync.dma_start(rsum_t[:], ar_out[:])
            rcp_all = small_pool.tile([P, NKt], f32, tag="rcp")
            nc.vector.reciprocal(rcp_all[:], rsum_t[:])

            bvb_t = small_pool.tile([P, D], f32, tag="bvb")
            nc.gpsimd.dma_start(bvb_t[:], bvb_d[:])

            # ---- phase V: V[kt][k, dv] = X@Wv + bv (full tokens; no rcp dep) ----
            v_tiles = []
            for kt in range(NKt):
                vt = v_pool.tile([P, D], bf16, tag=f"v{kt}", name=f"vt{kt}")
                v_tiles.append(vt)
            # Wv's second dv-half (if any) reuses QT slots (QT dead after K+S)
            for dvb in range(1, NDVB):
                for et in range(NE):
                    w = qt_pool.tile([P, NB], dt_in, tag=f"qt{et}", name=f"wv{et}_{dvb}")
                    nc.gpsimd.dma_start(w[:], wv_d[ts(et, P), ts(dvb, NB)])
                    wv_t[(et, dvb)] = w
            for kb in range(NKb):
                xv = []
                for et in range(NE):
                    t = x_pool.tile([P, NB], dt_in, tag="x", name=f"xv{kb}_{et}")
                    nc.sync.dma_start(t[:], xt_d[ts(et, P), ts(kb, NB)])
                    xv.append(t)
                for kt4 in range(KT_PER_B):
                    kt = kb * KT_PER_B + kt4
                    for dvb in range(NDVB):
                        ps = ps_pool.tile([P, NB], f32, tag="ps", name="psv")
                        for et in range(NE):
                            nc.tensor.matmul(
                                ps[:], r(xv[et][:, ts(kt4, P)]),
                                r(wv_t[(et, dvb)][:]),
                                start=(et == 0), stop=(et == NE - 1),
                            )
                        nc.vector.tensor_add(
                            v_tiles[kt][:, ts(dvb, NB)], ps[:], bvb_t[:, ts(dvb, NB)]
                        )

            # attn = PT * (1/D[k]) -- per-partition (k) scale, in place, on DVE
            for kt in range(NKt):
                nc.vector.tensor_scalar_mul(
                    pt_tiles[kt][:], pt_tiles[kt][:], rcp_all[:, kt:kt + 1]
                )

            # ---- phase AV: y[q, dv] = sum_k attn[k,q] * V[k,dv]; direct DMA out ----
            for qt in range(NQt):
                for dvb in range(NDVB):
                    ps = ps_pool.tile([P, NB], f32, tag="ps", name="psav")
                    for kt in range(NKt):
                        nc.tensor.matmul(
                            ps[:], pt_tiles[kt][:, ts(qt, P)],
                            v_tiles[kt][:, ts(dvb, NB)],
                            start=(kt == 0), stop=(kt == NKt - 1),
                        )
                    ob = out_pool.tile([P, NB], f32, tag="ob", name="ob")
                    nc.scalar.copy(ob[:], ps[:])
                    nc.sync.dma_start(y_d[ts(qt, P), ts(dvb, NB)], ob[:])

    nc.compile()
    return nc


def make_in_maps(cfg: Cfg, x, Wq, bq, Wk, bk, Wv, bv):
    SH = cfg.SH
    f32 = np.float32
    if cfg.mm == "bf16":
        import ml_dtypes
        dt_in = ml_dtypes.bfloat16
    else:
        dt_in = f32
    in_maps = []
    shared = {
        "wq": np.ascontiguousarray(Wq, dtype=dt_in),
        "wk": np.ascontiguousarray(Wk, dtype=dt_in),
        "wv": np.ascontiguousarray(Wv, dtype=dt_in),
        "bq": np.ascontiguousarray(np.reshape(bq, (-1, 1)), dtype=f32),
        "bk": np.ascontiguousarray(np.reshape(bk, (-1, 1)), dtype=f32),
        "bvb": np.ascontiguousarray(np.broadcast_to(np.reshape(bv, (1, -1)), (128, len(np.ravel(bv)))), dtype=f32),
    }
    for c in range(cfg.n_cores):
        b, h = c // 2, c % 2
        xb = np.asarray(x[b], dtype=f32)
        m = dict(shared)
        m["xt"] = np.ascontiguousarray(xb.T, dtype=dt_in)
        m["xth"] = np.ascontiguousarray(xb[h * SH:(h + 1) * SH, :].T, dtype=dt_in)
        in_maps.append(m)
    return in_maps


def run(inputs: dict, cfg: Cfg = PROD, trace: bool = False):
    from concourse.bass_utils import run_bass_kernel_spmd

    nc = build_nc(cfg)
    in_maps = make_in_maps(cfg, inputs["x"], inputs["Wq"], inputs["bq"],
                           inputs["Wk"], inputs["bk"], inputs["Wv"], inputs["bv"])
    res = run_bass_kernel_spmd(nc, in_maps, list(range(cfg.n_cores)), trace=trace)
    out = assemble(cfg, [r["y"] for r in res.results])
    return out, res


def assemble(cfg: Cfg, ys):
    """Core 2b holds q-rows [0, S/2), core 2b+1 holds [S/2, S) of batch b."""
    B, S, D = cfg.B, cfg.S, cfg.D
    out = np.empty((B, S, D), dtype=np.float32)
    for b in range(B):
        out[b, : cfg.SH] = ys[2 * b]
        out[b, cfg.SH:] = ys[2 * b + 1]
    return out


def kernel(**inputs) -> np.ndarray:
    out, _ = run(inputs, PROD, trace=False)
    return out


# revision 27
# speedup vs baseline: 1.3513x; 1.0086x over previous
"""Multi-head-free attention (softmax over the QUERY axis) on 8 trn2 NeuronCores.

Problem: x:[4,2048,1024], Wq/Wk/Wv:[1024,1024], bq/bk/bv:[1024]
    q = x@Wq+bq ; k = x@Wk+bk ; v = x@Wv+bv
    scores = einsum('bqd,bkd->bqk', q, k) / 32
    attn   = softmax(scores, axis=1)          # over q (dim 1)!
    out    = einsum('bqk,bkv->bqv', attn, v)

Sharding: 4 batches x 2-way split of the QUERY axis across 8 cores
(core c -> batch c//2, query-half c%2).  Each core computes its q-rows
of the output against the FULL key range, so the only cross-core
dependency is the softmax denominator D[k] = sum_q exp(s[q,k]) -- an
8 KB pairwise AllReduce that hides behind the V-projection phase.
There is no output collective: each core DMAs its q-rows directly.

All matmuls run as float32r (full PE rate at N=512 moving dim, fp32
storage).  The attn*V contraction runs in bf16 (attn weights + V), with
fp32 PSUM accumulation.
"""

import sys

if "/opt/trn_rl_repo" not in sys.path:
    sys.path.insert(0, "/opt/trn_rl_repo")

import numpy as np

P = 128  # SBUF partitions


class Cfg:
    def __init__(self, B=4, S=2048, E=1024, D=1024, NB=512, n_cores=8, mm="f32r"):
        self.B, self.S, self.E, self.D, self.NB = B, S, E, D, NB
        self.mm = mm
        self.SH = S // 2          # per-core query-half length
        self.NE = E // P          # e (contraction) tiles
        self.ND = D // P          # d tiles
        self.NQb = self.SH // NB  # q 512-blocks (my half)
        self.NKb = S // NB        # k 512-blocks (full)
        self.NKt = S // P         # k 128-tiles (full)
        self.NQt = self.SH // P   # q 128-tiles (my half)
        self.NDVB = D // NB       # dv 512-blocks
        self.n_cores = n_cores
        self.groups = [[2 * i, 2 * i + 1] for i in range(n_cores // 2)]


PROD = Cfg()


def build_nc(cfg: Cfg):
    from concourse import bacc, bass, mybir, tile

    f32 = mybir.dt.float32
    f32r = mybir.dt.float32r
    bf16 = mybir.dt.bfloat16
    AF = mybir.ActivationFunctionType
    X = mybir.AxisListType.X
    ts = bass.ts

    B, S, E, D, NB = cfg.B, cfg.S, cfg.E, cfg.D, cfg.NB
    SH, NE, ND = cfg.SH, cfg.NE, cfg.ND
    NQb, NKb, NKt, NQt, NDVB = cfg.NQb, cfg.NKb, cfg.NKt, cfg.NQt, cfg.NDVB
    KT_PER_B = NB // P
    inv_sqrt_d = 1.0 / float(np.sqrt(np.float32(D)))

    nc = bacc.Bacc(None, num_devices=cfg.n_cores)
    dt_in = bf16 if cfg.mm == "bf16" else f32r

    # Per-core inputs (host pre-shards / pre-transposes).
    # xt: X^T for the whole batch (K and V read all tokens).
    # xth: X^T columns of MY query-half (Q reads only those).
    xt_d = nc.declare_dram_parameter("xt", [E, S], dt_in, isOutput=False)
    xth_d = nc.declare_dram_parameter("xth", [E, SH], dt_in, isOutput=False)
    wq_d = nc.declare_dram_parameter("wq", [E, D], dt_in, isOutput=False)
    wk_d = nc.declare_dram_parameter("wk", [E, D], dt_in, isOutput=False)
    wv_d = nc.declare_dram_parameter("wv", [E, D], dt_in, isOutput=False)
    bq_d = nc.declare_dram_parameter("bq", [D, 1], f32, isOutput=False)
    bk_d = nc.declare_dram_parameter("bk", [D, 1], f32, isOutput=False)
    bvb_d = nc.declare_dram_parameter("bvb", [P, D], f32, isOutput=False)
    y_d = nc.declare_dram_parameter("y", [SH, D], f32, isOutput=True)

    def r(ap):  # tensors feeding the PE are already dt_in (f32r or bf16)
        return ap

    with tile.TileContext(nc) as tc:
        with (
            tc.tile_pool(name="w", bufs=NE + 1) as w_pool,
            tc.tile_pool(name="x", bufs=13) as x_pool,
            tc.tile_pool(name="qt", bufs=1) as qt_pool,
            tc.tile_pool(name="ktw", bufs=8) as ktw_pool,
            tc.tile_pool(name="pt", bufs=1) as pt_pool,
            tc.tile_pool(name="v", bufs=1) as v_pool,
            tc.tile_pool(name="wv", bufs=NE) as wv_pool,
            tc.tile_pool(name="ob", bufs=3) as out_pool,
            tc.tile_pool(name="small", bufs=1) as small_pool,
            tc.tile_pool(name="ps", bufs=8, space="PSUM") as ps_pool,
            tc.tile_pool(name="dram", bufs=1, space="DRAM") as dram_pool,
        ):
            ar_in = dram_pool.tile([P, NKt], f32)
            ar_out = dram_pool.tile([P, NKt], f32)

            # ---- prefetch Wq first (startup critical path), then constants ----
            wq_t = {}
            for et in range(NE):
                w = w_pool.tile([P, D], dt_in, tag="w", name=f"wq{et}")
                for hf in range(D // NB):
                    nc.gpsimd.dma_start(w[:, ts(hf, NB)], wq_d[ts(et, P), ts(hf, NB)])
                wq_t[et] = w

            # ---- constants (only bq is needed immediately) ----
            bq_t = []
            for dt in range(ND):
                bqt = small_pool.tile([P, 1], f32, tag=f"bq{dt}")
                nc.gpsimd.dma_start(bqt[:], bq_d[ts(dt, P), :])
                bq_t.append(bqt)

            # ---- phase Q: QT[dt][d, q] = Q[q, d]^T for MY q-half (resident) ----
            qt_tiles = []
            for dt in range(ND):
                q = qt_pool.tile([P, SH], dt_in, tag=f"qt{dt}", name=f"qtt{dt}")
                qt_tiles.append(q)
            for qb in range(NQb):
                xq = []
                for et in range(NE):
                    t = x_pool.tile([P, NB], dt_in, tag="x", name=f"xq{qb}_{et}")
                    nc.sync.dma_start(t[:], xth_d[ts(et, P), ts(qb, NB)])
                    xq.append(t)
                for dt in range(ND):
                    ps = ps_pool.tile([P, NB], f32, tag="ps", name="psq")
                    for et in range(NE):
                        nc.tensor.matmul(
                            ps[:], r(wq_t[et][:, ts(dt, P)]), r(xq[et][:]),
                            start=(et == 0), stop=(et == NE - 1),
                        )
                    nc.scalar.activation(
                        qt_tiles[dt][:, ts(qb, NB)], ps[:], AF.Identity, bias=bq_t[dt][:]
                    )

            # prefetch Wv's first dv-half early (bandwidth is idle during K+S)
            wv_t = {}
            for et in range(NE):
                w = wv_pool.tile([P, NB], dt_in, tag="wv", name=f"wv{et}_0")
                nc.gpsimd.dma_start(w[:], wv_d[ts(et, P), ts(0, NB)])
                wv_t[(et, 0)] = w

            bk_t = []
            for dt in range(ND):
                bkt = small_pool.tile([P, 1], f32, tag=f"bk{dt}")
                nc.gpsimd.dma_start(bkt[:], bk_d[ts(dt, P), :])
                bk_t.append(bkt)

            # ---- phase K+S fused: per 512-token k-block, project K then score ----
            # PT[kt][k, q] = exp(scores[q, k] / sqrt(D)) for all k, my q-half
            pt_tiles = []
            rs_t = []
            for kt in range(NKt):
                ptt = pt_pool.tile([P, SH], bf16, tag=f"pt{kt}", name=f"ptt{kt}")
                pt_tiles.append(ptt)
                rst = small_pool.tile([P, NQb], f32, tag=f"rs{kt}", name=f"rst{kt}")
                rs_t.append(rst)
            rs_all = small_pool.tile([P, NKt], f32, tag="rsall")
            wk_t = {}
            for kb in range(NKb):
                xk = []
                for et in range(NE):
                    t = x_pool.tile([P, NB], dt_in, tag="x", name=f"xk{kb}_{et}")
                    nc.sync.dma_start(t[:], xt_d[ts(et, P), ts(kb, NB)])
                    xk.append(t)
                if kb == 0:
                    for et in range(NE):
                        w = w_pool.tile([P, D], dt_in, tag="w", name=f"wk{et}")
                        for hf in range(D // NB):
                            nc.gpsimd.dma_start(w[:, ts(hf, NB)], wk_d[ts(et, P), ts(hf, NB)])
                        wk_t[et] = w
                ktw = []
                for dt in range(ND):
                    ps = ps_pool.tile([P, NB], f32, tag="ps", name="psk")
                    for et in range(NE):
                        nc.tensor.matmul(
                            ps[:], r(wk_t[et][:, ts(dt, P)]), r(xk[et][:]),
                            start=(et == 0), stop=(et == NE - 1),
                        )
                    kw = ktw_pool.tile([P, NB], dt_in, tag="ktw", name=f"ktw{kb}_{dt}")
                    nc.scalar.activation(kw[:], ps[:], AF.Identity, bias=bk_t[dt][:])
                    ktw.append(kw)
                for kt4 in range(KT_PER_B):
                    kt = kb * KT_PER_B + kt4
                    for qb in range(NQb):
                        ps = ps_pool.tile([P, NB], f32, tag="ps", name="pss")
                        for dt in range(ND):
                            nc.tensor.matmul(
                                ps[:], r(ktw[dt][:, ts(kt4, P)]),
                                r(qt_tiles[dt][:, ts(qb, NB)]),
                                start=(dt == 0), stop=(dt == ND - 1),
                            )
                        nc.scalar.activation(
                            pt_tiles[kt][:, ts(qb, NB)], ps[:], AF.Exp,
                            scale=inv_sqrt_d,
                            accum_out=rs_t[kt][:, qb:qb + 1],
                        )
                    nc.vector.reduce_sum(rs_all[:, kt:kt + 1], rs_t[kt][:], axis=X)

            # ---- softmax denominators: pairwise 8KB AllReduce, then 1/D ----
            nc.sync.dma_start(ar_in[:], rs_all[:])
            nc.gpsimd.collective_compute(
                "AllReduce",
                mybir.AluOpType.add,
                replica_groups=cfg.groups,
                ins=[ar_in[:].opt()],
                outs=[ar_out[:].opt()],
            )
            rsum_t = small_pool.tile([P, NKt], f32, tag="rsum")
            nc.sync.dma_start(rsum_t[:], ar_out[:])
            rcp_all = small_pool.tile([P, NKt], f32, tag="rcp")
            nc.vector.reciprocal(rcp_all[:], rsum_t[:])

            bvb_t = small_pool.tile([P, D], f32, tag="bvb")
            nc.scalar.dma_start(bvb_t[:], bvb_d[:])

            # ---- phase V: V[kt][k, dv] = X@Wv + bv (full tokens; no rcp dep) ----
            v_tiles = []
            for kt in range(NKt):
                vt = v_pool.tile([P, D], bf16, tag=f"v{kt}", name=f"vt{kt}")
                v_tiles.append(vt)
            # Wv's second dv-half (if any) reuses QT slots (QT dead after K+S)
            for dvb in range(1, NDVB):
                for et in range(NE):
                    w = qt_pool.tile([P, NB], dt_in, tag=f"qt{et}", name=f"wv{et}_{dvb}")
                    nc.scalar.dma_start(w[:], wv_d[ts(et, P), ts(dvb, NB)])
                    wv_t[(et, dvb)] = w
            for kb in range(NKb):
                xv = []
                for et in range(NE):
                    t = x_pool.tile([P, NB], dt_in, tag="x", name=f"xv{kb}_{et}")
                    nc.sync.dma_start(t[:], xt_d[ts(et, P), ts(kb, NB)])
                    xv.append(t)
                for kt4 in range(KT_PER_B):
                    kt = kb * KT_PER_B + kt4
                    for dvb in range(NDVB):
                        ps = ps_pool.tile([P, NB], f32, tag="ps", name="psv")
                        for et in range(NE):
                            nc.tensor.matmul(
                                ps[:], r(xv[et][:, ts(kt4, P)]),
                                r(wv_t[(et, dvb)][:]),
                                start=(et == 0), stop=(et == NE - 1),
                            )
                        nc.vector.tensor_add(
                            v_tiles[kt][:, ts(dvb, NB)], ps[:], bvb_t[:, ts(dvb, NB)]
                        )

            # attn = PT * (1/D[k]) -- per-partition (k) scale, in place, on DVE
            for kt in range(NKt):
                nc.vector.tensor_scalar_mul(
                    pt_tiles[kt][:], pt_tiles[kt][:], rcp_all[:, kt:kt + 1]
                )

            # ---- phase AV: y[q, dv] = sum_k attn[k,q] * V[k,dv]; direct DMA out ----
            for qt in range(NQt):
                for dvb in range(NDVB):
                    ps = ps_pool.tile([P, NB], f32, tag="ps", name="psav")
                    for kt in range(NKt):
                        nc.tensor.matmul(
                            ps[:], pt_tiles[kt][:, ts(qt, P)],
                            v_tiles[kt][:, ts(dvb, NB)],
                            start=(kt == 0), stop=(kt == NKt - 1),
                        )
                    ob = out_pool.tile([P, NB], f32, tag="ob", name="ob")
                    nc.scalar.copy(ob[:], ps[:])
                    eng = nc.sync if (qt + dvb) % 2 == 0 else nc.scalar
                    eng.dma_start(y_d[ts(qt, P), ts(dvb, NB)], ob[:])

    nc.compile()
    return nc


def make_in_maps(cfg: Cfg, x, Wq, bq, Wk, bk, Wv, bv):
    SH = cfg.SH
    f32 = np.float32
    if cfg.mm == "bf16":
        import ml_dtypes
        dt_in = ml_dtypes.bfloat16
    else:
        dt_in = f32
    in_maps = []
    shared = {
        "wq": np.ascontiguousarray(Wq, dtype=dt_in),
        "wk": np.ascontiguousarray(Wk, dtype=dt_in),
        "wv": np.ascontiguousarray(Wv, dtype=dt_in),
        "bq": np.ascontiguousarray(np.reshape(bq, (-1, 1)), dtype=f32),
        "bk": np.ascontiguousarray(np.reshape(bk, (-1, 1)), dtype=f32),
        "bvb": np.ascontiguousarray(np.broadcast_to(np.reshape(bv, (1, -1)), (128, len(np.ravel(bv)))), dtype=f32),
    }
    for c in range(cfg.n_cores):
        b, h = c // 2, c % 2
        xb = np.asarray(x[b], dtype=f32)
        m = dict(shared)
        m["xt"] = np.ascontiguousarray(xb.T, dtype=dt_in)
        m["xth"] = np.ascontiguousarray(xb[h * SH:(h + 1) * SH, :].T, dtype=dt_in)
        in_maps.append(m)
    return in_maps


def run(inputs: dict, cfg: Cfg = PROD, trace: bool = False):
    from concourse.bass_utils import run_bass_kernel_spmd

    nc = build_nc(cfg)
    in_maps = make_in_maps(cfg, inputs["x"], inputs["Wq"], inputs["bq"],
                           inputs["Wk"], inputs["bk"], inputs["Wv"], inputs["bv"])
    res = run_bass_kernel_spmd(nc, in_maps, list(range(cfg.n_cores)), trace=trace)
    out = assemble(cfg, [r["y"] for r in res.results])
    return out, res


def assemble(cfg: Cfg, ys):
    """Core 2b holds q-rows [0, S/2), core 2b+1 holds [S/2, S) of batch b."""
    B, S, D = cfg.B, cfg.S, cfg.D
    out = np.empty((B, S, D), dtype=np.float32)
    for b in range(B):
        out[b, : cfg.SH] = ys[2 * b]
        out[b, cfg.SH:] = ys[2 * b + 1]
    return out


def kernel(**inputs) -> np.ndarray:
    out, _ = run(inputs, PROD, trace=False)
    return out
